# revision 1
# baseline (speedup 1.0000x reference)
"""DeepCross kernel for 8x TRN2 NeuronCores.

Math: the cross-network keeps temp = x0 * f with f a per-row scalar, so the
whole model collapses to G = x0 @ [cross_w | w1 | wf_x0]  ([B, 37]) plus a
tiny per-row tail:
    g = G[:, :4]; p1 = G[:, 4:36]; q = G[:, 36:37]
    f1 = 1 + g0 + b0; f2 = f1*g1 + b1; f3 = f2*(1+g2) + b2; f4 = f3*g3 + b3
    h1 = relu(f4 * p1); h2 = relu(h1 @ w2); out = sigmoid(h2 @ wf_h + q + bf)

Device strategy (data-parallel over batch, 1024 rows/core):
  - emb table quad-packed to bf16 [25000, 512B]; dma_gather with idx = x//4
    (fits int16) in slot-major order lands quads at [batch_part, slot, 256].
  - 1-of-4 sub-row select via 3 predicated copies with host-built masks.
  - PE-transpose 128x128 chunks, accumulate G^T [37, 128] per subtile on PE.
  - Tail on 1..37-partition tiles; one [1, 1024] f32 row DMA'd out per core.
"""
import sys
sys.path.insert(0, '/opt/trn_rl_repo')
import os
import numpy as np
import ml_dtypes

from concourse import bass, mybir
import concourse.tile as tile
from concourse import bacc, library_config
from concourse.bass_utils import run_bass_kernel_spmd
from concourse.masks import make_identity
from concourse.tile import add_dep_helper

BF16 = ml_dtypes.bfloat16

B, T, E = 8192, 128, 64
V = 100000
D = T * E                 # 8192
L = 4
H1, H2 = 32, 16
NCORES = 8
BC = B // NCORES          # 1024 batch rows per core
NSUB = BC // 128          # 8 subtiles of 128 rows
GS = 64                   # slots per gather group (half a subtile's 128 slots)
NGRP = NSUB * 2           # 16 gather groups per core
NIDX = 128 * GS           # 8192 indices per gather
QE = 256                  # bf16 elements per quad row (512 B)
UQ = V // 4               # 25000 quad rows
NCHUNK = D // 128         # 64 d-chunks per subtile
NW = L + H1 + 1           # 37 fused weight columns

_PROGRAM = None
KMODE = os.environ.get('KMODE', 'full')


def _build_program():
    f32 = mybir.dt.float32
    bf16 = mybir.dt.bfloat16
    nc = bacc.Bacc("TRN2", target_bir_lowering=False, debug=False,
                   num_devices=NCORES, dynamic_dma_scratch_size=32768)

    tblq = nc.dram_tensor("tblq", [UQ, QE], bf16, kind="ExternalInput")
    xidx = nc.dram_tensor("xidx", [128, NGRP * (NIDX // 16)], mybir.dt.int16,
                          kind="ExternalInput")
    xmask = nc.dram_tensor("xmask", [128, NGRP * 3 * GS], mybir.dt.uint8,
                           kind="ExternalInput")
    wbd = nc.dram_tensor("wb", [128, NCHUNK * NW], bf16, kind="ExternalInput")
    w2d = nc.dram_tensor("w2", [H1, H2], f32, kind="ExternalInput")
    wfhd = nc.dram_tensor("wfh", [H2, 1], f32, kind="ExternalInput")
    wfh2d = nc.dram_tensor("wfh2", [H1, 2], f32, kind="ExternalInput")
    cbd = nc.dram_tensor("cb", [1, L], f32, kind="ExternalInput")
    b1d = nc.dram_tensor("b1v", [1, H1], f32, kind="ExternalInput")
    b2d = nc.dram_tensor("b2v", [1, H2], f32, kind="ExternalInput")
    bfd = nc.dram_tensor("bfv", [1, 1], f32, kind="ExternalInput")
    outd = nc.dram_tensor("out", [NSUB, 128], f32, kind="ExternalOutput")

    AF = mybir.ActivationFunctionType
    OP = mybir.AluOpType

    with tile.TileContext(nc) as tc:
        with (
            tc.tile_pool(name="const", bufs=1) as cpool,
            tc.tile_pool(name="io", bufs=3) as iopool,
            tc.tile_pool(name="quad", bufs=3) as qpool,
            tc.tile_pool(name="x0c", bufs=3) as xpool,
            tc.tile_pool(name="xt", bufs=4) as xtpool,
            tc.tile_pool(name="tail", bufs=2) as sp,
            tc.tile_pool(name="ptp", bufs=4, space="PSUM") as ptpool,
            tc.tile_pool(name="pgt", bufs=2, space="PSUM") as pgpool,
            tc.tile_pool(name="pts", bufs=2, space="PSUM") as pspool,
        ):
            nc.gpsimd.load_library(library_config.mlp)

            wb_t = cpool.tile([128, NCHUNK * NW], bf16)
            nc.sync.dma_start(out=wb_t[:], in_=wbd.ap())
            ident = cpool.tile([128, 128], bf16)
            make_identity(nc, ident[:])
            ident32 = cpool.tile([128, 128], f32)
            make_identity(nc, ident32[:])
            w2_t = cpool.tile([H1, H2], f32)
            nc.sync.dma_start(out=w2_t[:], in_=w2d.ap())
            wfh_t = cpool.tile([H2, 1], f32)
            nc.sync.dma_start(out=wfh_t[:], in_=wfhd.ap())
            wfh2_t = cpool.tile([H1, 2], f32)
            nc.sync.dma_start(out=wfh2_t[:], in_=wfh2d.ap())
            # pack [cb(4) | b1(32) | b2(16) | bf(1)] into one row, broadcast
            # to all 128 partitions via a k=1 matmul with a ones column.
            NPK = L + H1 + H2 + 1
            pack_t = cpool.tile([1, NPK], f32)
            nc.sync.dma_start(out=pack_t[0:1, 0:L], in_=cbd.ap())
            nc.sync.dma_start(out=pack_t[0:1, L:L + H1], in_=b1d.ap())
            nc.sync.dma_start(out=pack_t[0:1, L + H1:L + H1 + H2], in_=b2d.ap())
            nc.sync.dma_start(out=pack_t[0:1, L + H1 + H2:NPK], in_=bfd.ap())
            ones_r = cpool.tile([1, 128], f32)
            nc.vector.memset(ones_r[:], 1.0)
            packb_p = pspool.tile([128, NPK], f32, tag="tps")
            packb_mm = nc.tensor.matmul(out=packb_p[:], lhsT=ones_r[:],
                                        rhs=pack_t[:], start=True, stop=True)
            prev_tail_pe0 = packb_mm.ins
            packb = cpool.tile([128, NPK], f32)
            nc.vector.tensor_copy(out=packb[:], in_=packb_p[:])
            cbb = packb[:, 0:L]
            b1b = packb[:, L:L + H1]
            b2b = packb[:, L + H1:L + H1 + H2]
            bfb = packb[:, L + H1 + H2:NPK]
            out_col = None
            if KMODE != "notail":
                out_col = cpool.tile([128, NSUB], f32, tag="out_col")
            gts_all = cpool.tile([NW, BC], f32, tag="gts_all")
            idx_all = cpool.tile([128, NGRP * (NIDX // 16)], mybir.dt.int16,
                                 tag="idx_all")
            nc.sync.dma_start(out=idx_all[:], in_=xidx.ap())
            msk_all = cpool.tile([128, NGRP * 3 * GS], mybir.dt.uint8,
                                 tag="msk_all")
            nc.sync.dma_start(out=msk_all[:], in_=xmask.ap())

            prev_tail_pe = prev_tail_pe0
            for sub in range(NSUB):
                gt = pgpool.tile([NW, 128], f32, tag="gt")
                for h in range(2):
                    g = sub * 2 + h
                    idx_t = idx_all[:, g * (NIDX // 16):(g + 1) * (NIDX // 16)]
                    msk_t = msk_all[:, g * 3 * GS:(g + 1) * 3 * GS]

                    quad = qpool.tile([128, GS * QE], bf16, tag="quad")
                    qview = quad[:].rearrange("p (s e) -> p s e", e=QE)
                    nsplit = 2
                    step = GS // nsplit
                    for sp_i in range(nsplit):
                        nc.gpsimd.dma_gather(
                            out_ap=qview[:, sp_i * step:(sp_i + 1) * step, :],
                            in_ap=tblq.ap(),
                            idxs_ap=idx_t[:, sp_i * (step * 128 // 16):
                                          (sp_i + 1) * (step * 128 // 16)],
                            num_idxs=128 * step,
                            num_idxs_reg=128 * step,
                            elem_size=QE,
                            single_packet=False,
                        )

                    x0c = xpool.tile([128, GS * 64], bf16, tag="x0c")
                    bsel = xpool.tile([128, GS * 64], bf16, tag="bsel")
                    qv = quad[:].rearrange("p (s e) -> p s e", e=QE)
                    xv = x0c[:].rearrange("p (s e) -> p s e", e=64)
                    bv = bsel[:].rearrange("p (s e) -> p s e", e=64)
                    HG = GS // 2

                    def _mk(i, lo, hi):
                        m = msk_t[:, i * GS + lo:i * GS + hi]
                        m = m.rearrange("p (s one) -> p s one", one=1)
                        return m.to_broadcast([128, hi - lo, 64])

                    for lo in (0, HG):
                        hi = lo + HG
                        nc.scalar.copy(out=xv[:, lo:hi, :],
                                       in_=qv[:, lo:hi, 0:64])
                        nc.scalar.copy(out=bv[:, lo:hi, :],
                                       in_=qv[:, lo:hi, 128:192])
                        nc.vector.copy_predicated(out=xv[:, lo:hi, :],
                                                  mask=_mk(0, lo, hi),
                                                  data=qv[:, lo:hi, 64:128])
                        nc.vector.copy_predicated(out=bv[:, lo:hi, :],
                                                  mask=_mk(1, lo, hi),
                                                  data=qv[:, lo:hi, 192:256])
                        nc.vector.copy_predicated(out=xv[:, lo:hi, :],
                                                  mask=_mk(2, lo, hi),
                                                  data=bv[:, lo:hi, :])
                    for c4 in range(GS // 8):   # 8 groups of 4 chunks
                        tp = ptpool.tile([128, 512], bf16, tag="tp")
                        for j in range(4):
                            c2 = c4 * 4 + j
                            nc.tensor.transpose(
                                out=tp[:, j * 128:(j + 1) * 128],
                                in_=x0c[:, c2 * 128:(c2 + 1) * 128],
                                identity=ident[:],
                            )
                        xt = xtpool.tile([128, 512], bf16, tag="xt")
                        nc.scalar.copy(out=xt[:], in_=tp[:])
                        for j in range(4):
                            cd = h * (GS // 2) + c4 * 4 + j
                            mm = nc.tensor.matmul(
                                out=gt[:],
                                lhsT=wb_t[:, cd * NW:(cd + 1) * NW],
                                rhs=xt[:, j * 128:(j + 1) * 128],
                                start=(cd == 0),
                                stop=(cd == NCHUNK - 1),
                            )
                            if cd == 0 and sub == 0 and prev_tail_pe is not None:
                                add_dep_helper(mm.ins, prev_tail_pe,
                                               reason="packb before accum groups")

                # stash G^T for the batched tail at the end
                nc.scalar.copy(out=gts_all[:, sub * 128:(sub + 1) * 128],
                               in_=gt[:])
                if KMODE == "notail" and sub == 0:
                    nc.sync.dma_start(out=outd.ap(), in_=gts_all[0:NSUB, 0:128])

            if KMODE != "notail":
                # ---- batched tail over all 8 subtiles ----
                gtt_all = cpool.tile([128, NSUB * NW], f32, tag="gtt_all")
                for s in range(NSUB):
                    gtt_p = pspool.tile([128, 64], f32, tag="tps")
                    nc.tensor.transpose(
                        out=gtt_p[:, 0:NW],
                        in_=gts_all[:, s * 128:(s + 1) * 128],
                        identity=ident32[0:NW, 0:NW])
                    nc.scalar.copy(out=gtt_all[:, s * NW:(s + 1) * NW],
                                   in_=gtt_p[:, 0:NW])
                gv = gtt_all[:].rearrange("p (s w) -> p s w", w=NW)

                # f-recurrence on [128, NSUB] column groups
                f1 = cpool.tile([128, NSUB], f32, tag="f1")
                nc.vector.tensor_scalar(out=f1[:], in0=gv[:, :, 0:1],
                                        scalar1=cbb[:, 0:1], scalar2=1.0,
                                        op0=OP.add, op1=OP.add)
                f2 = cpool.tile([128, NSUB], f32, tag="f2")
                nc.vector.tensor_tensor(out=f2[:], in0=f1[:].rearrange(
                    "p (s one) -> p s one", one=1), in1=gv[:, :, 1:2], op=OP.mult)
                nc.vector.tensor_scalar(out=f2[:], in0=f2[:],
                                        scalar1=cbb[:, 1:2], scalar2=None,
                                        op0=OP.add)
                u3 = cpool.tile([128, NSUB], f32, tag="u3")
                nc.vector.tensor_scalar(out=u3[:], in0=gv[:, :, 2:3],
                                        scalar1=1.0, scalar2=None, op0=OP.add)
                f3 = cpool.tile([128, NSUB], f32, tag="f3")
                nc.vector.tensor_tensor(out=f3[:], in0=f2[:], in1=u3[:],
                                        op=OP.mult)
                nc.vector.tensor_scalar(out=f3[:], in0=f3[:],
                                        scalar1=cbb[:, 2:3], scalar2=None,
                                        op0=OP.add)
                f4 = cpool.tile([128, NSUB], f32, tag="f4")
                nc.vector.tensor_tensor(out=f4[:], in0=f3[:].rearrange(
                    "p (s one) -> p s one", one=1), in1=gv[:, :, 3:4], op=OP.mult)
                nc.vector.tensor_scalar(out=f4[:], in0=f4[:],
                                        scalar1=cbb[:, 3:4], scalar2=None,
                                        op0=OP.add)

                # h1 = relu(f4 * p1 + b1)  [128, NSUB, H1]
                h1_all = cpool.tile([128, NSUB * H1], f32, tag="h1_all")
                h1v = h1_all[:].rearrange("p (s h) -> p s h", h=H1)
                nc.vector.tensor_tensor(
                    out=h1v, in0=gv[:, :, L:L + H1],
                    in1=f4[:].rearrange("p (s one) -> p s one", one=1)
                        .to_broadcast([128, NSUB, H1]),
                    op=OP.mult)
                nc.vector.tensor_tensor(
                    out=h1v, in0=h1v,
                    in1=b1b.rearrange("p (one h) -> p one h", one=1)
                        .to_broadcast([128, NSUB, H1]),
                    op=OP.add)
                nc.scalar.activation(out=h1_all[:], in_=h1_all[:], func=AF.Relu)

                # h2 = relu(h1 @ w2 + b2): pack 4 subs per [128,128] transpose
                h2_all = cpool.tile([128, NSUB * H2], f32, tag="h2_all")
                for q4 in range(NSUB // 4):
                    h1T_p = pspool.tile([128, 128], f32, tag="tps")
                    nc.tensor.transpose(
                        out=h1T_p[:],
                        in_=h1_all[:, q4 * 4 * H1:(q4 + 1) * 4 * H1],
                        identity=ident32[:])
                    h1T = cpool.tile([128, 128], f32, tag="h1T")
                    nc.scalar.copy(out=h1T[:], in_=h1T_p[:])
                    for j in range(4):
                        s = q4 * 4 + j
                        h1Tj = cpool.tile([H1, 128], f32, tag="h1Tj")
                        nc.scalar.copy(out=h1Tj[:],
                                       in_=h1T[j * H1:(j + 1) * H1, :])
                        h2p = pspool.tile([128, H2], f32, tag="tps")
                        nc.tensor.matmul(out=h2p[:], lhsT=h1Tj[:],
                                         rhs=w2_t[:], start=True, stop=True)
                        nc.scalar.copy(out=h2_all[:, s * H2:(s + 1) * H2],
                                       in_=h2p[:])
                h2v = h2_all[:].rearrange("p (s h) -> p s h", h=H2)
                nc.vector.tensor_tensor(
                    out=h2v, in0=h2v,
                    in1=b2b.rearrange("p (one h) -> p one h", one=1)
                        .to_broadcast([128, NSUB, H2]),
                    op=OP.add)
                nc.scalar.activation(out=h2_all[:], in_=h2_all[:], func=AF.Relu)

                # z = h2 @ wf_h ; out = sigmoid(z + q + bf)
                z_all = cpool.tile([128, NSUB], f32, tag="z_all")
                h2T_p = pspool.tile([128, 128], f32, tag="tps")
                nc.tensor.transpose(out=h2T_p[:], in_=h2_all[:],
                                    identity=ident32[:])
                h2T = cpool.tile([128, 128], f32, tag="h2T")
                nc.scalar.copy(out=h2T[:], in_=h2T_p[:])
                for pr in range(NSUB // 2):
                    h2Tp2 = cpool.tile([2 * H2, 128], f32, tag="h2Tp2")
                    nc.scalar.copy(out=h2Tp2[:],
                                   in_=h2T[pr * 2 * H2:(pr + 1) * 2 * H2, :])
                    zp = pspool.tile([128, 2], f32, tag="tps")
                    nc.tensor.matmul(out=zp[:], lhsT=h2Tp2[:],
                                     rhs=wfh2_t[:], start=True, stop=True)
                    nc.scalar.copy(out=z_all[:, pr * 2:(pr + 1) * 2], in_=zp[:])
                nc.vector.tensor_tensor(out=z_all[:], in0=z_all[:].rearrange(
                    "p (s one) -> p s one", one=1), in1=gv[:, :, NW - 1:NW],
                    op=OP.add)
                nc.scalar.activation(out=out_col[:], in_=z_all[:],
                                     func=AF.Sigmoid,
                                     bias=bfb[:, 0:1], scale=1.0)

                oT_p = pspool.tile([NSUB, 128], f32, tag="tps")
                nc.tensor.transpose(out=oT_p[:], in_=out_col[:],
                                    identity=ident32[:])
                oT = cpool.tile([NSUB, 128], f32)
                nc.scalar.copy(out=oT[:], in_=oT_p[:])
                nc.sync.dma_start(out=outd.ap(), in_=oT[:])

    nc.compile()
    return nc


def _get_program():
    global _PROGRAM
    if _PROGRAM is None:
        _PROGRAM = _build_program()
    return _PROGRAM


def _host_prep(x, emb, cross_w, cross_b, w1, b1, w2, b2, wf, bf):
    x = np.asarray(x)
    emb = np.ascontiguousarray(np.asarray(emb, dtype=np.float32))
    cross_w = np.asarray(cross_w, dtype=np.float32)
    cross_b = np.asarray(cross_b, dtype=np.float32)
    w1 = np.asarray(w1, dtype=np.float32)
    w2 = np.asarray(w2, dtype=np.float32)
    b1 = np.asarray(b1, dtype=np.float32)
    b2 = np.asarray(b2, dtype=np.float32)
    wf = np.asarray(wf, dtype=np.float32)
    bf = np.asarray(bf, dtype=np.float32)

    tblq = emb.astype(BF16).reshape(UQ, QE)
    wbig = np.concatenate([cross_w[:, :, 0].T, w1, wf[H2:, :]], axis=1)  # [D, 37]
    wb_np = np.ascontiguousarray(
        wbig.reshape(NCHUNK, 128, NW).transpose(1, 0, 2).reshape(128, NCHUNK * NW)
    ).astype(BF16)

    shared = {
        "tblq": tblq,
        "wb": wb_np,
        "w2": w2,
        "wfh": np.ascontiguousarray(wf[:H2, :]),
        "wfh2": np.asarray(np.block([[wf[:H2, :], np.zeros((H2, 1), np.float32)], [np.zeros((H2, 1), np.float32), wf[:H2, :]]]), dtype=np.float32),
        "cb": cross_b.reshape(1, L),
        "b1v": b1.reshape(1, H1),
        "b2v": b2.reshape(1, H2),
        "bfv": bf.reshape(1, 1),
    }

    in_maps = []
    for c in range(NCORES):
        xc = x[c * BC:(c + 1) * BC].astype(np.int64)
        xq = (xc // 4).astype(np.int16)          # [1024, 128]
        xr = (xc % 4).astype(np.int8)
        idx_np = np.empty((NGRP, 128, NIDX // 16), dtype=np.int16)
        msk_np = np.empty((NGRP, 128, 3 * GS), dtype=np.uint8)
        for g in range(NGRP):
            s, hh = g // 2, g % 2
            blk = xq[s * 128:(s + 1) * 128, hh * GS:(hh + 1) * GS]  # [128b, 64t]
            lst = blk.T.reshape(-1)                                 # i = t*128+b
            idx_np[g] = np.tile(lst.reshape(NIDX // 16, 16).T, (8, 1))
            rb = xr[s * 128:(s + 1) * 128, hh * GS:(hh + 1) * GS]
            msk_np[g, :, 0 * GS:1 * GS] = (rb % 2 == 1).astype(np.uint8)
            msk_np[g, :, 1 * GS:2 * GS] = (rb == 3).astype(np.uint8)
            msk_np[g, :, 2 * GS:3 * GS] = (rb >= 2).astype(np.uint8)
        m = dict(shared)
        m["xidx"] = np.ascontiguousarray(idx_np.transpose(1, 0, 2).reshape(128, -1))
        m["xmask"] = np.ascontiguousarray(msk_np.transpose(1, 0, 2).reshape(128, -1))
        in_maps.append(m)
    return in_maps


def _ensure_ntff_hook():
    """The image's antenv lacks axon_hooks; synthesize it so
    run_bass_kernel_spmd(trace=True) can NTFF-profile via the axon .so."""
    import types
    if 'antenv.axon_hooks' in sys.modules:
        return
    import antenv
    mod = types.ModuleType('antenv.axon_hooks')
    _state = {'hook': None}
    def set_axon_ntff_profile_hook(h):
        _state['hook'] = h
    def get_axon_ntff_profile_hook():
        if _state['hook'] is None:
            try:
                from trn_agent_boot.trn_boot import _ntff_profile_via_ctypes
                _state['hook'] = _ntff_profile_via_ctypes('/opt/axon/libaxon_pjrt.so')
            except Exception:
                return None
        return _state['hook']
    mod.set_axon_ntff_profile_hook = set_axon_ntff_profile_hook
    mod.get_axon_ntff_profile_hook = get_axon_ntff_profile_hook
    sys.modules['antenv.axon_hooks'] = mod
    antenv.axon_hooks = mod


def run(inputs: dict, trace: bool = False):
    if trace:
        _ensure_ntff_hook()
    nc = _get_program()
    in_maps = _host_prep(**inputs)
    res = run_bass_kernel_spmd(nc, in_maps, core_ids=list(range(NCORES)),
                               trace=trace)
    out = np.concatenate(
        [np.asarray(res.results[c]["out"]).reshape(BC, 1) for c in range(NCORES)]
    )
    return out.astype(np.float32), res


def kernel(**inputs):
    out, _ = run(inputs, trace=False)
    return out



# revision 4
# speedup vs baseline: 2.4104x; 2.4104x over previous
"""DeepCross kernel for 8x TRN2 NeuronCores.

Math: the cross-network keeps temp = x0 * f with f a per-row scalar, so the
whole model collapses to G = x0 @ [cross_w | w1 | wf_x0]  ([B, 37]) plus a
tiny per-row tail:
    g = G[:, :4]; p1 = G[:, 4:36]; q = G[:, 36:37]
    f1 = 1 + g0 + b0; f2 = f1*g1 + b1; f3 = f2*(1+g2) + b2; f4 = f3*g3 + b3
    h1 = relu(f4 * p1); h2 = relu(h1 @ w2); out = sigmoid(h2 @ wf_h + q + bf)

Device strategy (data-parallel over batch, 1024 rows/core):
  - emb table quad-packed to bf16 [25000, 512B]; dma_gather with idx = x//4
    (fits int16) in slot-major order lands quads at [batch_part, slot, 256].
  - 1-of-4 sub-row select via 3 predicated copies with host-built masks.
  - PE-transpose 128x128 chunks, accumulate G^T [37, 128] per subtile on PE.
  - Tail on 1..37-partition tiles; one [1, 1024] f32 row DMA'd out per core.
"""
import sys
sys.path.insert(0, '/opt/trn_rl_repo')
import os
import numpy as np
import ml_dtypes

from concourse import bass, mybir
import concourse.tile as tile
from concourse import bacc, library_config
from concourse.bass_utils import run_bass_kernel_spmd
from concourse.masks import make_identity
from concourse.tile import add_dep_helper

BF16 = ml_dtypes.bfloat16

B, T, E = 8192, 128, 64
V = 100000
D = T * E                 # 8192
L = 4
H1, H2 = 32, 16
NCORES = 8
BC = B // NCORES          # 1024 batch rows per core
NSUB = BC // 128          # 8 subtiles of 128 rows
GS = 64                   # slots per gather group (half a subtile's 128 slots)
NGRP = NSUB * 2           # 16 gather groups per core
NIDX = 128 * GS           # 8192 indices per gather
QE = 256                  # bf16 elements per quad row (512 B)
UQ = V // 4               # 25000 quad rows
NCHUNK = D // 128         # 64 d-chunks per subtile
NW = L + H1 + 1           # 37 fused weight columns

_PROGRAM = None
KMODE = os.environ.get('KMODE', 'full')
NSWQ = int(os.environ.get('NSWQ', '4'))     # SWDGE queues (1..4)


def _build_program():
    f32 = mybir.dt.float32
    bf16 = mybir.dt.bfloat16
    nc = bacc.Bacc("TRN2", target_bir_lowering=False, debug=False,
                   num_devices=NCORES, dynamic_dma_scratch_size=32768,
                   num_swdge_queues=NSWQ)

    tblq = nc.dram_tensor("tblq", [UQ, QE], bf16, kind="ExternalInput")
    xidx = nc.dram_tensor("xidx", [128, NGRP * (NIDX // 16)], mybir.dt.int16,
                          kind="ExternalInput")
    xmask = nc.dram_tensor("xmask", [128, NGRP * 3 * GS], mybir.dt.uint8,
                           kind="ExternalInput")
    wbd = nc.dram_tensor("wb", [128, NCHUNK * NW], bf16, kind="ExternalInput")
    w2d = nc.dram_tensor("w2", [H1, H2], f32, kind="ExternalInput")
    wfhd = nc.dram_tensor("wfh", [H2, 1], f32, kind="ExternalInput")
    wfh2d = nc.dram_tensor("wfh2", [H1, 2], f32, kind="ExternalInput")
    cbd = nc.dram_tensor("cb", [1, L], f32, kind="ExternalInput")
    b1d = nc.dram_tensor("b1v", [1, H1], f32, kind="ExternalInput")
    b2d = nc.dram_tensor("b2v", [1, H2], f32, kind="ExternalInput")
    bfd = nc.dram_tensor("bfv", [1, 1], f32, kind="ExternalInput")
    outd = nc.dram_tensor("out", [NSUB, 128], f32, kind="ExternalOutput")

    AF = mybir.ActivationFunctionType
    OP = mybir.AluOpType

    with tile.TileContext(nc) as tc:
        with (
            tc.tile_pool(name="const", bufs=1) as cpool,
            tc.tile_pool(name="io", bufs=3) as iopool,
            tc.tile_pool(name="quad", bufs=3) as qpool,
            tc.tile_pool(name="x0c", bufs=3) as xpool,
            tc.tile_pool(name="xt", bufs=4) as xtpool,
            tc.tile_pool(name="tail", bufs=2) as sp,
            tc.tile_pool(name="ptp", bufs=4, space="PSUM") as ptpool,
            tc.tile_pool(name="pgt", bufs=2, space="PSUM") as pgpool,
            tc.tile_pool(name="pts", bufs=2, space="PSUM") as pspool,
        ):
            nc.gpsimd.load_library(library_config.mlp)

            wb_t = cpool.tile([128, NCHUNK * NW], bf16)
            nc.sync.dma_start(out=wb_t[:], in_=wbd.ap())
            ident = cpool.tile([128, 128], bf16)
            make_identity(nc, ident[:])
            ident32 = cpool.tile([128, 128], f32)
            make_identity(nc, ident32[:])
            w2_t = cpool.tile([H1, H2], f32)
            nc.sync.dma_start(out=w2_t[:], in_=w2d.ap())
            wfh_t = cpool.tile([H2, 1], f32)
            nc.sync.dma_start(out=wfh_t[:], in_=wfhd.ap())
            wfh2_t = cpool.tile([H1, 2], f32)
            nc.sync.dma_start(out=wfh2_t[:], in_=wfh2d.ap())
            # pack [cb(4) | b1(32) | b2(16) | bf(1)] into one row, broadcast
            # to all 128 partitions via a k=1 matmul with a ones column.
            NPK = L + H1 + H2 + 1
            pack_t = cpool.tile([1, NPK], f32)
            nc.sync.dma_start(out=pack_t[0:1, 0:L], in_=cbd.ap())
            nc.sync.dma_start(out=pack_t[0:1, L:L + H1], in_=b1d.ap())
            nc.sync.dma_start(out=pack_t[0:1, L + H1:L + H1 + H2], in_=b2d.ap())
            nc.sync.dma_start(out=pack_t[0:1, L + H1 + H2:NPK], in_=bfd.ap())
            ones_r = cpool.tile([1, 128], f32)
            nc.vector.memset(ones_r[:], 1.0)
            packb_p = pspool.tile([128, NPK], f32, tag="tps")
            packb_mm = nc.tensor.matmul(out=packb_p[:], lhsT=ones_r[:],
                                        rhs=pack_t[:], start=True, stop=True)
            prev_tail_pe0 = packb_mm.ins
            packb = cpool.tile([128, NPK], f32)
            nc.vector.tensor_copy(out=packb[:], in_=packb_p[:])
            cbb = packb[:, 0:L]
            b1b = packb[:, L:L + H1]
            b2b = packb[:, L + H1:L + H1 + H2]
            bfb = packb[:, L + H1 + H2:NPK]
            out_col = None
            if KMODE != "notail":
                out_col = cpool.tile([128, NSUB], f32, tag="out_col")
            gts_all = cpool.tile([NW, BC], f32, tag="gts_all")
            idx_all = cpool.tile([128, NGRP * (NIDX // 16)], mybir.dt.int16,
                                 tag="idx_all")
            nc.sync.dma_start(out=idx_all[:], in_=xidx.ap())
            msk_all = cpool.tile([128, NGRP * 3 * GS], mybir.dt.uint8,
                                 tag="msk_all")
            nc.sync.dma_start(out=msk_all[:], in_=xmask.ap())

            prev_tail_pe = prev_tail_pe0
            for sub in range(NSUB):
                gt = pgpool.tile([NW, 128], f32, tag="gt")
                for h in range(2):
                    g = sub * 2 + h
                    idx_t = idx_all[:, g * (NIDX // 16):(g + 1) * (NIDX // 16)]
                    msk_t = msk_all[:, g * 3 * GS:(g + 1) * 3 * GS]

                    quad = qpool.tile([128, GS * QE], bf16, tag="quad")
                    qview = quad[:].rearrange("p (s e) -> p s e", e=QE)
                    nsplit = 2
                    step = GS // nsplit
                    for sp_i in range(nsplit):
                        nc.gpsimd.dma_gather(
                            out_ap=qview[:, sp_i * step:(sp_i + 1) * step, :],
                            in_ap=tblq.ap(),
                            idxs_ap=idx_t[:, sp_i * (step * 128 // 16):
                                          (sp_i + 1) * (step * 128 // 16)],
                            num_idxs=128 * step,
                            num_idxs_reg=128 * step,
                            elem_size=QE,
                            single_packet=False,
                            queue_num=(g * nsplit + sp_i) % NSWQ,
                        )

                    x0c = xpool.tile([128, GS * 64], bf16, tag="x0c")
                    bsel = xpool.tile([128, GS * 64], bf16, tag="bsel")
                    qv = quad[:].rearrange("p (s e) -> p s e", e=QE)
                    xv = x0c[:].rearrange("p (s e) -> p s e", e=64)
                    bv = bsel[:].rearrange("p (s e) -> p s e", e=64)
                    HG = GS // 2

                    def _mk(i, lo, hi):
                        m = msk_t[:, i * GS + lo:i * GS + hi]
                        m = m.rearrange("p (s one) -> p s one", one=1)
                        return m.to_broadcast([128, hi - lo, 64])

                    for lo in (0, HG):
                        hi = lo + HG
                        nc.scalar.copy(out=xv[:, lo:hi, :],
                                       in_=qv[:, lo:hi, 0:64])
                        nc.scalar.copy(out=bv[:, lo:hi, :],
                                       in_=qv[:, lo:hi, 128:192])
                        nc.vector.copy_predicated(out=xv[:, lo:hi, :],
                                                  mask=_mk(0, lo, hi),
                                                  data=qv[:, lo:hi, 64:128])
                        nc.vector.copy_predicated(out=bv[:, lo:hi, :],
                                                  mask=_mk(1, lo, hi),
                                                  data=qv[:, lo:hi, 192:256])
                        nc.vector.copy_predicated(out=xv[:, lo:hi, :],
                                                  mask=_mk(2, lo, hi),
                                                  data=bv[:, lo:hi, :])
                    for c4 in range(GS // 8):   # 8 groups of 4 chunks
                        tp = ptpool.tile([128, 512], bf16, tag="tp")
                        for j in range(4):
                            c2 = c4 * 4 + j
                            nc.tensor.transpose(
                                out=tp[:, j * 128:(j + 1) * 128],
                                in_=x0c[:, c2 * 128:(c2 + 1) * 128],
                                identity=ident[:],
                            )
                        xt = xtpool.tile([128, 512], bf16, tag="xt")
                        nc.scalar.copy(out=xt[:], in_=tp[:])
                        for j in range(4):
                            cd = h * (GS // 2) + c4 * 4 + j
                            mm = nc.tensor.matmul(
                                out=gt[:],
                                lhsT=wb_t[:, cd * NW:(cd + 1) * NW],
                                rhs=xt[:, j * 128:(j + 1) * 128],
                                start=(cd == 0),
                                stop=(cd == NCHUNK - 1),
                            )
                            if cd == 0 and sub == 0 and prev_tail_pe is not None:
                                add_dep_helper(mm.ins, prev_tail_pe,
                                               reason="packb before accum groups")

                # stash G^T for the batched tail at the end
                nc.scalar.copy(out=gts_all[:, sub * 128:(sub + 1) * 128],
                               in_=gt[:])
                if KMODE == "notail" and sub == 0:
                    nc.sync.dma_start(out=outd.ap(), in_=gts_all[0:NSUB, 0:128])

            if KMODE != "notail":
                # ---- batched tail over all 8 subtiles ----
                gtt_all = cpool.tile([128, NSUB * NW], f32, tag="gtt_all")
                for s in range(NSUB):
                    gtt_p = pspool.tile([128, 64], f32, tag="tps")
                    nc.tensor.transpose(
                        out=gtt_p[:, 0:NW],
                        in_=gts_all[:, s * 128:(s + 1) * 128],
                        identity=ident32[0:NW, 0:NW])
                    nc.scalar.copy(out=gtt_all[:, s * NW:(s + 1) * NW],
                                   in_=gtt_p[:, 0:NW])
                gv = gtt_all[:].rearrange("p (s w) -> p s w", w=NW)

                # f-recurrence on [128, NSUB] column groups
                f1 = cpool.tile([128, NSUB], f32, tag="f1")
                nc.vector.tensor_scalar(out=f1[:], in0=gv[:, :, 0:1],
                                        scalar1=cbb[:, 0:1], scalar2=1.0,
                                        op0=OP.add, op1=OP.add)
                f2 = cpool.tile([128, NSUB], f32, tag="f2")
                nc.vector.tensor_tensor(out=f2[:], in0=f1[:].rearrange(
                    "p (s one) -> p s one", one=1), in1=gv[:, :, 1:2], op=OP.mult)
                nc.vector.tensor_scalar(out=f2[:], in0=f2[:],
                                        scalar1=cbb[:, 1:2], scalar2=None,
                                        op0=OP.add)
                u3 = cpool.tile([128, NSUB], f32, tag="u3")
                nc.vector.tensor_scalar(out=u3[:], in0=gv[:, :, 2:3],
                                        scalar1=1.0, scalar2=None, op0=OP.add)
                f3 = cpool.tile([128, NSUB], f32, tag="f3")
                nc.vector.tensor_tensor(out=f3[:], in0=f2[:], in1=u3[:],
                                        op=OP.mult)
                nc.vector.tensor_scalar(out=f3[:], in0=f3[:],
                                        scalar1=cbb[:, 2:3], scalar2=None,
                                        op0=OP.add)
                f4 = cpool.tile([128, NSUB], f32, tag="f4")
                nc.vector.tensor_tensor(out=f4[:], in0=f3[:].rearrange(
                    "p (s one) -> p s one", one=1), in1=gv[:, :, 3:4], op=OP.mult)
                nc.vector.tensor_scalar(out=f4[:], in0=f4[:],
                                        scalar1=cbb[:, 3:4], scalar2=None,
                                        op0=OP.add)

                # h1 = relu(f4 * p1 + b1)  [128, NSUB, H1]
                h1_all = cpool.tile([128, NSUB * H1], f32, tag="h1_all")
                h1v = h1_all[:].rearrange("p (s h) -> p s h", h=H1)
                nc.vector.tensor_tensor(
                    out=h1v, in0=gv[:, :, L:L + H1],
                    in1=f4[:].rearrange("p (s one) -> p s one", one=1)
                        .to_broadcast([128, NSUB, H1]),
                    op=OP.mult)
                nc.vector.tensor_tensor(
                    out=h1v, in0=h1v,
                    in1=b1b.rearrange("p (one h) -> p one h", one=1)
                        .to_broadcast([128, NSUB, H1]),
                    op=OP.add)
                nc.scalar.activation(out=h1_all[:], in_=h1_all[:], func=AF.Relu)

                # h2 = relu(h1 @ w2 + b2): pack 4 subs per [128,128] transpose
                h2_all = cpool.tile([128, NSUB * H2], f32, tag="h2_all")
                for q4 in range(NSUB // 4):
                    h1T_p = pspool.tile([128, 128], f32, tag="tps")
                    nc.tensor.transpose(
                        out=h1T_p[:],
                        in_=h1_all[:, q4 * 4 * H1:(q4 + 1) * 4 * H1],
                        identity=ident32[:])
                    h1T = cpool.tile([128, 128], f32, tag="h1T")
                    nc.scalar.copy(out=h1T[:], in_=h1T_p[:])
                    for j in range(4):
                        s = q4 * 4 + j
                        h1Tj = cpool.tile([H1, 128], f32, tag="h1Tj")
                        nc.scalar.copy(out=h1Tj[:],
                                       in_=h1T[j * H1:(j + 1) * H1, :])
                        h2p = pspool.tile([128, H2], f32, tag="tps")
                        nc.tensor.matmul(out=h2p[:], lhsT=h1Tj[:],
                                         rhs=w2_t[:], start=True, stop=True)
                        nc.scalar.copy(out=h2_all[:, s * H2:(s + 1) * H2],
                                       in_=h2p[:])
                h2v = h2_all[:].rearrange("p (s h) -> p s h", h=H2)
                nc.vector.tensor_tensor(
                    out=h2v, in0=h2v,
                    in1=b2b.rearrange("p (one h) -> p one h", one=1)
                        .to_broadcast([128, NSUB, H2]),
                    op=OP.add)
                nc.scalar.activation(out=h2_all[:], in_=h2_all[:], func=AF.Relu)

                # z = h2 @ wf_h ; out = sigmoid(z + q + bf)
                z_all = cpool.tile([128, NSUB], f32, tag="z_all")
                h2T_p = pspool.tile([128, 128], f32, tag="tps")
                nc.tensor.transpose(out=h2T_p[:], in_=h2_all[:],
                                    identity=ident32[:])
                h2T = cpool.tile([128, 128], f32, tag="h2T")
                nc.scalar.copy(out=h2T[:], in_=h2T_p[:])
                for pr in range(NSUB // 2):
                    h2Tp2 = cpool.tile([2 * H2, 128], f32, tag="h2Tp2")
                    nc.scalar.copy(out=h2Tp2[:],
                                   in_=h2T[pr * 2 * H2:(pr + 1) * 2 * H2, :])
                    zp = pspool.tile([128, 2], f32, tag="tps")
                    nc.tensor.matmul(out=zp[:], lhsT=h2Tp2[:],
                                     rhs=wfh2_t[:], start=True, stop=True)
                    nc.scalar.copy(out=z_all[:, pr * 2:(pr + 1) * 2], in_=zp[:])
                nc.vector.tensor_tensor(out=z_all[:], in0=z_all[:].rearrange(
                    "p (s one) -> p s one", one=1), in1=gv[:, :, NW - 1:NW],
                    op=OP.add)
                nc.scalar.activation(out=out_col[:], in_=z_all[:],
                                     func=AF.Sigmoid,
                                     bias=bfb[:, 0:1], scale=1.0)

                oT_p = pspool.tile([NSUB, 128], f32, tag="tps")
                nc.tensor.transpose(out=oT_p[:], in_=out_col[:],
                                    identity=ident32[:])
                oT = cpool.tile([NSUB, 128], f32)
                nc.scalar.copy(out=oT[:], in_=oT_p[:])
                nc.sync.dma_start(out=outd.ap(), in_=oT[:])

    nc.compile()
    return nc


def _get_program():
    global _PROGRAM
    if _PROGRAM is None:
        _PROGRAM = _build_program()
    return _PROGRAM


def _host_prep(x, emb, cross_w, cross_b, w1, b1, w2, b2, wf, bf):
    x = np.asarray(x)
    emb = np.ascontiguousarray(np.asarray(emb, dtype=np.float32))
    cross_w = np.asarray(cross_w, dtype=np.float32)
    cross_b = np.asarray(cross_b, dtype=np.float32)
    w1 = np.asarray(w1, dtype=np.float32)
    w2 = np.asarray(w2, dtype=np.float32)
    b1 = np.asarray(b1, dtype=np.float32)
    b2 = np.asarray(b2, dtype=np.float32)
    wf = np.asarray(wf, dtype=np.float32)
    bf = np.asarray(bf, dtype=np.float32)

    tblq = emb.astype(BF16).reshape(UQ, QE)
    wbig = np.concatenate([cross_w[:, :, 0].T, w1, wf[H2:, :]], axis=1)  # [D, 37]
    wb_np = np.ascontiguousarray(
        wbig.reshape(NCHUNK, 128, NW).transpose(1, 0, 2).reshape(128, NCHUNK * NW)
    ).astype(BF16)

    shared = {
        "tblq": tblq,
        "wb": wb_np,
        "w2": w2,
        "wfh": np.ascontiguousarray(wf[:H2, :]),
        "wfh2": np.asarray(np.block([[wf[:H2, :], np.zeros((H2, 1), np.float32)], [np.zeros((H2, 1), np.float32), wf[:H2, :]]]), dtype=np.float32),
        "cb": cross_b.reshape(1, L),
        "b1v": b1.reshape(1, H1),
        "b2v": b2.reshape(1, H2),
        "bfv": bf.reshape(1, 1),
    }

    in_maps = []
    for c in range(NCORES):
        xc = x[c * BC:(c + 1) * BC].astype(np.int64)
        xq = (xc // 4).astype(np.int16)          # [1024, 128]
        xr = (xc % 4).astype(np.int8)
        idx_np = np.empty((NGRP, 128, NIDX // 16), dtype=np.int16)
        msk_np = np.empty((NGRP, 128, 3 * GS), dtype=np.uint8)
        for g in range(NGRP):
            s, hh = g // 2, g % 2
            blk = xq[s * 128:(s + 1) * 128, hh * GS:(hh + 1) * GS]  # [128b, 64t]
            lst = blk.T.reshape(-1)                                 # i = t*128+b
            idx_np[g] = np.tile(lst.reshape(NIDX // 16, 16).T, (8, 1))
            rb = xr[s * 128:(s + 1) * 128, hh * GS:(hh + 1) * GS]
            msk_np[g, :, 0 * GS:1 * GS] = (rb % 2 == 1).astype(np.uint8)
            msk_np[g, :, 1 * GS:2 * GS] = (rb == 3).astype(np.uint8)
            msk_np[g, :, 2 * GS:3 * GS] = (rb >= 2).astype(np.uint8)
        m = dict(shared)
        m["xidx"] = np.ascontiguousarray(idx_np.transpose(1, 0, 2).reshape(128, -1))
        m["xmask"] = np.ascontiguousarray(msk_np.transpose(1, 0, 2).reshape(128, -1))
        in_maps.append(m)
    return in_maps


def _ensure_ntff_hook():
    """The image's antenv lacks axon_hooks; synthesize it so
    run_bass_kernel_spmd(trace=True) can NTFF-profile via the axon .so."""
    import types
    if 'antenv.axon_hooks' in sys.modules:
        return
    import antenv
    mod = types.ModuleType('antenv.axon_hooks')
    _state = {'hook': None}
    def set_axon_ntff_profile_hook(h):
        _state['hook'] = h
    def get_axon_ntff_profile_hook():
        if _state['hook'] is None:
            try:
                from trn_agent_boot.trn_boot import _ntff_profile_via_ctypes
                _state['hook'] = _ntff_profile_via_ctypes('/opt/axon/libaxon_pjrt.so')
            except Exception:
                return None
        return _state['hook']
    mod.set_axon_ntff_profile_hook = set_axon_ntff_profile_hook
    mod.get_axon_ntff_profile_hook = get_axon_ntff_profile_hook
    sys.modules['antenv.axon_hooks'] = mod
    antenv.axon_hooks = mod


def run(inputs: dict, trace: bool = False):
    if trace:
        _ensure_ntff_hook()
    nc = _get_program()
    in_maps = _host_prep(**inputs)
    res = run_bass_kernel_spmd(nc, in_maps, core_ids=list(range(NCORES)),
                               trace=trace)
    out = np.concatenate(
        [np.asarray(res.results[c]["out"]).reshape(BC, 1) for c in range(NCORES)]
    )
    return out.astype(np.float32), res


def kernel(**inputs):
    out, _ = run(inputs, trace=False)
    return out



# revision 8
# speedup vs baseline: 3.0238x; 1.2545x over previous
"""DeepCross kernel for 8x TRN2 NeuronCores.

Math: the cross-network keeps temp = x0 * f with f a per-row scalar, so the
whole model collapses to G = x0 @ [cross_w | w1 | wf_x0]  ([B, 37]) plus a
tiny per-row tail:
    g = G[:, :4]; p1 = G[:, 4:36]; q = G[:, 36:37]
    f1 = 1 + g0 + b0; f2 = f1*g1 + b1; f3 = f2*(1+g2) + b2; f4 = f3*g3 + b3
    h1 = relu(f4 * p1); h2 = relu(h1 @ w2); out = sigmoid(h2 @ wf_h + q + bf)

Device strategy (data-parallel over batch, 1024 rows/core):
  - emb table quad-packed to bf16 [25000, 512B]; dma_gather with idx = x//4
    (fits int16) in slot-major order lands quads at [batch_part, slot, 256].
  - 1-of-4 sub-row select via 3 predicated copies with host-built masks.
  - PE-transpose 128x128 chunks, accumulate G^T [37, 128] per subtile on PE.
  - Tail on 1..37-partition tiles; one [1, 1024] f32 row DMA'd out per core.
"""
import sys
sys.path.insert(0, '/opt/trn_rl_repo')
import os
import numpy as np
import ml_dtypes

from concourse import bass, mybir
import concourse.tile as tile
from concourse import bacc, library_config
from concourse.bass_utils import run_bass_kernel_spmd
from concourse.masks import make_identity
from concourse.tile import add_dep_helper

BF16 = ml_dtypes.bfloat16

B, T, E = 8192, 128, 64
V = 100000
D = T * E                 # 8192
L = 4
H1, H2 = 32, 16
NCORES = 8
BC = B // NCORES          # 1024 batch rows per core
NSUB = BC // 128          # 8 subtiles of 128 rows
GS = 32                   # slots per gather group (quarter subtile)
GPS = 4                   # groups per subtile
NGRP = NSUB * GPS         # 32 gather groups per core
NIDX = 128 * GS           # 4096 indices per gather
QE = 256                  # bf16 elements per quad row (512 B)
UQ = V // 4               # 25000 quad rows
NCHUNK = D // 128         # 64 d-chunks per subtile
NW = L + H1 + 1           # 37 fused weight columns

_PROGRAM = None
KMODE = os.environ.get('KMODE', 'full')
NSWQ = int(os.environ.get('NSWQ', '4'))     # SWDGE queues (1..4)


def _build_program():
    f32 = mybir.dt.float32
    bf16 = mybir.dt.bfloat16
    nc = bacc.Bacc("TRN2", target_bir_lowering=False, debug=False,
                   num_devices=NCORES, dynamic_dma_scratch_size=32768,
                   num_swdge_queues=NSWQ)

    tblq = nc.dram_tensor("tblq", [UQ, QE], bf16, kind="ExternalInput")
    xidx = nc.dram_tensor("xidx", [128, NGRP * (NIDX // 16)], mybir.dt.int16,
                          kind="ExternalInput")
    xmask = nc.dram_tensor("xmask", [128, NGRP * 3 * GS], mybir.dt.uint8,
                           kind="ExternalInput")
    wbd = nc.dram_tensor("wb", [128, NCHUNK * NW], bf16, kind="ExternalInput")
    w2d = nc.dram_tensor("w2", [H1, H2], f32, kind="ExternalInput")
    wfhd = nc.dram_tensor("wfh", [H2, 1], f32, kind="ExternalInput")
    wfh2d = nc.dram_tensor("wfh2", [H1, 2], f32, kind="ExternalInput")
    cbd = nc.dram_tensor("cb", [1, L], f32, kind="ExternalInput")
    b1d = nc.dram_tensor("b1v", [1, H1], f32, kind="ExternalInput")
    b2d = nc.dram_tensor("b2v", [1, H2], f32, kind="ExternalInput")
    bfd = nc.dram_tensor("bfv", [1, 1], f32, kind="ExternalInput")
    outd = nc.dram_tensor("out", [NSUB, 128], f32, kind="ExternalOutput")

    AF = mybir.ActivationFunctionType
    OP = mybir.AluOpType

    with tile.TileContext(nc) as tc:
        with (
            tc.tile_pool(name="const", bufs=1) as cpool,
            tc.tile_pool(name="io", bufs=3) as iopool,
            tc.tile_pool(name="quad", bufs=6) as qpool,
            tc.tile_pool(name="x0c", bufs=4) as xpool,
            tc.tile_pool(name="xt", bufs=4) as xtpool,
            tc.tile_pool(name="tail", bufs=2) as sp,
            tc.tile_pool(name="ptp", bufs=4, space="PSUM") as ptpool,
            tc.tile_pool(name="pgt", bufs=2, space="PSUM") as pgpool,
            tc.tile_pool(name="pts", bufs=2, space="PSUM") as pspool,
        ):
            nc.gpsimd.load_library(library_config.mlp)

            wb_t = cpool.tile([128, NCHUNK * NW], bf16)
            nc.sync.dma_start(out=wb_t[:], in_=wbd.ap())
            ident = cpool.tile([128, 128], bf16)
            make_identity(nc, ident[:])
            ident32 = cpool.tile([128, 128], f32)
            make_identity(nc, ident32[:])
            w2_t = cpool.tile([H1, H2], f32)
            nc.sync.dma_start(out=w2_t[:], in_=w2d.ap())
            wfh_t = cpool.tile([H2, 1], f32)
            nc.sync.dma_start(out=wfh_t[:], in_=wfhd.ap())
            wfh2_t = cpool.tile([H1, 2], f32)
            nc.sync.dma_start(out=wfh2_t[:], in_=wfh2d.ap())
            # pack [cb(4) | b1(32) | b2(16) | bf(1)] into one row, broadcast
            # to all 128 partitions via a k=1 matmul with a ones column.
            NPK = L + H1 + H2 + 1
            pack_t = cpool.tile([1, NPK], f32)
            nc.sync.dma_start(out=pack_t[0:1, 0:L], in_=cbd.ap())
            nc.sync.dma_start(out=pack_t[0:1, L:L + H1], in_=b1d.ap())
            nc.sync.dma_start(out=pack_t[0:1, L + H1:L + H1 + H2], in_=b2d.ap())
            nc.sync.dma_start(out=pack_t[0:1, L + H1 + H2:NPK], in_=bfd.ap())
            ones_r = cpool.tile([1, 128], f32)
            nc.vector.memset(ones_r[:], 1.0)
            packb_p = pspool.tile([128, NPK], f32, tag="tps")
            packb_mm = nc.tensor.matmul(out=packb_p[:], lhsT=ones_r[:],
                                        rhs=pack_t[:], start=True, stop=True)
            prev_tail_pe0 = packb_mm.ins
            packb = cpool.tile([128, NPK], f32)
            nc.vector.tensor_copy(out=packb[:], in_=packb_p[:])
            cbb = packb[:, 0:L]
            b1b = packb[:, L:L + H1]
            b2b = packb[:, L + H1:L + H1 + H2]
            bfb = packb[:, L + H1 + H2:NPK]
            out_col = None
            if KMODE != "notail":
                out_col = cpool.tile([128, NSUB], f32, tag="out_col")
            gts_all = cpool.tile([NW, BC], f32, tag="gts_all")
            idx_all = cpool.tile([128, NGRP * (NIDX // 16)], mybir.dt.int16,
                                 tag="idx_all")
            nc.sync.dma_start(out=idx_all[:], in_=xidx.ap())
            msk_all = cpool.tile([128, NGRP * 3 * GS], mybir.dt.uint8,
                                 tag="msk_all")
            nc.sync.dma_start(out=msk_all[:], in_=xmask.ap())

            prev_tail_pe = prev_tail_pe0
            for sub in range(NSUB):
                gt = pgpool.tile([NW, 128], f32, tag="gt")
                for h in range(GPS):
                    g = sub * GPS + h
                    idx_t = idx_all[:, g * (NIDX // 16):(g + 1) * (NIDX // 16)]
                    msk_t = msk_all[:, g * 3 * GS:(g + 1) * 3 * GS]

                    quad = qpool.tile([128, GS * QE], bf16, tag="quad")
                    qview = quad[:].rearrange("p (s e) -> p s e", e=QE)
                    nc.gpsimd.dma_gather(
                        out_ap=qview[:, :, :],
                        in_ap=tblq.ap(),
                        idxs_ap=idx_t[:],
                        num_idxs=NIDX,
                        num_idxs_reg=NIDX,
                        elem_size=QE,
                        single_packet=False,
                        queue_num=g % NSWQ,
                    )

                    x0c = xpool.tile([128, GS * 64], bf16, tag="x0c")
                    qv32 = quad[:].bitcast(f32).rearrange("p (s e) -> p s e",
                                                          e=QE // 2)
                    xv32 = x0c[:].bitcast(f32).rearrange("p (s e) -> p s e",
                                                         e=32)
                    HG = GS // 2

                    def _mk(i, lo, hi):
                        m = msk_t[:, i * GS + lo:i * GS + hi]
                        m = m.rearrange("p (s one) -> p s one", one=1)
                        return m.to_broadcast([128, hi - lo, 32])

                    for lo in (0, HG):
                        hi = lo + HG
                        nc.scalar.copy(out=xv32[:, lo:hi, :],
                                       in_=qv32[:, lo:hi, 0:32])
                        nc.vector.copy_predicated(out=xv32[:, lo:hi, :],
                                                  mask=_mk(0, lo, hi),
                                                  data=qv32[:, lo:hi, 32:64])
                        nc.vector.copy_predicated(out=xv32[:, lo:hi, :],
                                                  mask=_mk(1, lo, hi),
                                                  data=qv32[:, lo:hi, 64:96])
                        nc.vector.copy_predicated(out=xv32[:, lo:hi, :],
                                                  mask=_mk(2, lo, hi),
                                                  data=qv32[:, lo:hi, 96:128])
                    for c4 in range(GS // 8):   # groups of 4 chunks
                        tp = ptpool.tile([128, 512], bf16, tag="tp")
                        for j in range(4):
                            c2 = c4 * 4 + j
                            nc.tensor.transpose(
                                out=tp[:, j * 128:(j + 1) * 128],
                                in_=x0c[:, c2 * 128:(c2 + 1) * 128],
                                identity=ident[:],
                            )
                        xt = xtpool.tile([128, 512], bf16, tag="xt")
                        nc.scalar.copy(out=xt[:], in_=tp[:])
                        for j in range(4):
                            cd = h * (GS // 2) + c4 * 4 + j
                            mm = nc.tensor.matmul(
                                out=gt[:],
                                lhsT=wb_t[:, cd * NW:(cd + 1) * NW],
                                rhs=xt[:, j * 128:(j + 1) * 128],
                                start=(cd == 0),
                                stop=(cd == NCHUNK - 1),
                            )
                            if cd == 0 and sub == 0 and prev_tail_pe is not None:
                                add_dep_helper(mm.ins, prev_tail_pe,
                                               reason="packb before accum groups")

                # stash G^T for the batched tail at the end
                nc.scalar.copy(out=gts_all[:, sub * 128:(sub + 1) * 128],
                               in_=gt[:])
                if KMODE == "notail" and sub == 0:
                    nc.sync.dma_start(out=outd.ap(), in_=gts_all[0:NSUB, 0:128])

            if KMODE != "notail":
                # ---- batched tail over all 8 subtiles ----
                gtt_all = cpool.tile([128, NSUB * NW], f32, tag="gtt_all")
                for s in range(NSUB):
                    gtt_p = pspool.tile([128, 64], f32, tag="tps")
                    nc.tensor.transpose(
                        out=gtt_p[:, 0:NW],
                        in_=gts_all[:, s * 128:(s + 1) * 128],
                        identity=ident32[0:NW, 0:NW])
                    nc.scalar.copy(out=gtt_all[:, s * NW:(s + 1) * NW],
                                   in_=gtt_p[:, 0:NW])
                gv = gtt_all[:].rearrange("p (s w) -> p s w", w=NW)

                # f-recurrence on [128, NSUB] column groups
                f1 = cpool.tile([128, NSUB], f32, tag="f1")
                nc.vector.tensor_scalar(out=f1[:], in0=gv[:, :, 0:1],
                                        scalar1=cbb[:, 0:1], scalar2=1.0,
                                        op0=OP.add, op1=OP.add)
                f2 = cpool.tile([128, NSUB], f32, tag="f2")
                nc.vector.tensor_tensor(out=f2[:], in0=f1[:].rearrange(
                    "p (s one) -> p s one", one=1), in1=gv[:, :, 1:2], op=OP.mult)
                nc.vector.tensor_scalar(out=f2[:], in0=f2[:],
                                        scalar1=cbb[:, 1:2], scalar2=None,
                                        op0=OP.add)
                u3 = cpool.tile([128, NSUB], f32, tag="u3")
                nc.vector.tensor_scalar(out=u3[:], in0=gv[:, :, 2:3],
                                        scalar1=1.0, scalar2=None, op0=OP.add)
                f3 = cpool.tile([128, NSUB], f32, tag="f3")
                nc.vector.tensor_tensor(out=f3[:], in0=f2[:], in1=u3[:],
                                        op=OP.mult)
                nc.vector.tensor_scalar(out=f3[:], in0=f3[:],
                                        scalar1=cbb[:, 2:3], scalar2=None,
                                        op0=OP.add)
                f4 = cpool.tile([128, NSUB], f32, tag="f4")
                nc.vector.tensor_tensor(out=f4[:], in0=f3[:].rearrange(
                    "p (s one) -> p s one", one=1), in1=gv[:, :, 3:4], op=OP.mult)
                nc.vector.tensor_scalar(out=f4[:], in0=f4[:],
                                        scalar1=cbb[:, 3:4], scalar2=None,
                                        op0=OP.add)

                # h1 = relu(f4 * p1 + b1)  [128, NSUB, H1]
                h1_all = cpool.tile([128, NSUB * H1], f32, tag="h1_all")
                h1v = h1_all[:].rearrange("p (s h) -> p s h", h=H1)
                nc.vector.tensor_tensor(
                    out=h1v, in0=gv[:, :, L:L + H1],
                    in1=f4[:].rearrange("p (s one) -> p s one", one=1)
                        .to_broadcast([128, NSUB, H1]),
                    op=OP.mult)
                nc.vector.tensor_tensor(
                    out=h1v, in0=h1v,
                    in1=b1b.rearrange("p (one h) -> p one h", one=1)
                        .to_broadcast([128, NSUB, H1]),
                    op=OP.add)
                nc.scalar.activation(out=h1_all[:], in_=h1_all[:], func=AF.Relu)

                # h2 = relu(h1 @ w2 + b2): pack 4 subs per [128,128] transpose
                h2_all = cpool.tile([128, NSUB * H2], f32, tag="h2_all")
                for q4 in range(NSUB // 4):
                    h1T_p = pspool.tile([128, 128], f32, tag="tps")
                    nc.tensor.transpose(
                        out=h1T_p[:],
                        in_=h1_all[:, q4 * 4 * H1:(q4 + 1) * 4 * H1],
                        identity=ident32[:])
                    h1T = cpool.tile([128, 128], f32, tag="h1T")
                    nc.scalar.copy(out=h1T[:], in_=h1T_p[:])
                    for j in range(4):
                        s = q4 * 4 + j
                        h1Tj = cpool.tile([H1, 128], f32, tag="h1Tj")
                        nc.scalar.copy(out=h1Tj[:],
                                       in_=h1T[j * H1:(j + 1) * H1, :])
                        h2p = pspool.tile([128, H2], f32, tag="tps")
                        nc.tensor.matmul(out=h2p[:], lhsT=h1Tj[:],
                                         rhs=w2_t[:], start=True, stop=True)
                        nc.scalar.copy(out=h2_all[:, s * H2:(s + 1) * H2],
                                       in_=h2p[:])
                h2v = h2_all[:].rearrange("p (s h) -> p s h", h=H2)
                nc.vector.tensor_tensor(
                    out=h2v, in0=h2v,
                    in1=b2b.rearrange("p (one h) -> p one h", one=1)
                        .to_broadcast([128, NSUB, H2]),
                    op=OP.add)
                nc.scalar.activation(out=h2_all[:], in_=h2_all[:], func=AF.Relu)

                # z = h2 @ wf_h ; out = sigmoid(z + q + bf)
                z_all = cpool.tile([128, NSUB], f32, tag="z_all")
                h2T_p = pspool.tile([128, 128], f32, tag="tps")
                nc.tensor.transpose(out=h2T_p[:], in_=h2_all[:],
                                    identity=ident32[:])
                h2T = cpool.tile([128, 128], f32, tag="h2T")
                nc.scalar.copy(out=h2T[:], in_=h2T_p[:])
                for pr in range(NSUB // 2):
                    h2Tp2 = cpool.tile([2 * H2, 128], f32, tag="h2Tp2")
                    nc.scalar.copy(out=h2Tp2[:],
                                   in_=h2T[pr * 2 * H2:(pr + 1) * 2 * H2, :])
                    zp = pspool.tile([128, 2], f32, tag="tps")
                    nc.tensor.matmul(out=zp[:], lhsT=h2Tp2[:],
                                     rhs=wfh2_t[:], start=True, stop=True)
                    nc.scalar.copy(out=z_all[:, pr * 2:(pr + 1) * 2], in_=zp[:])
                nc.vector.tensor_tensor(out=z_all[:], in0=z_all[:].rearrange(
                    "p (s one) -> p s one", one=1), in1=gv[:, :, NW - 1:NW],
                    op=OP.add)
                nc.scalar.activation(out=out_col[:], in_=z_all[:],
                                     func=AF.Sigmoid,
                                     bias=bfb[:, 0:1], scale=1.0)

                oT_p = pspool.tile([NSUB, 128], f32, tag="tps")
                nc.tensor.transpose(out=oT_p[:], in_=out_col[:],
                                    identity=ident32[:])
                oT = cpool.tile([NSUB, 128], f32)
                nc.scalar.copy(out=oT[:], in_=oT_p[:])
                nc.sync.dma_start(out=outd.ap(), in_=oT[:])

    nc.compile()
    return nc


def _get_program():
    global _PROGRAM
    if _PROGRAM is None:
        _PROGRAM = _build_program()
    return _PROGRAM


def _host_prep(x, emb, cross_w, cross_b, w1, b1, w2, b2, wf, bf):
    x = np.asarray(x)
    emb = np.ascontiguousarray(np.asarray(emb, dtype=np.float32))
    cross_w = np.asarray(cross_w, dtype=np.float32)
    cross_b = np.asarray(cross_b, dtype=np.float32)
    w1 = np.asarray(w1, dtype=np.float32)
    w2 = np.asarray(w2, dtype=np.float32)
    b1 = np.asarray(b1, dtype=np.float32)
    b2 = np.asarray(b2, dtype=np.float32)
    wf = np.asarray(wf, dtype=np.float32)
    bf = np.asarray(bf, dtype=np.float32)

    tblq = emb.astype(BF16).reshape(UQ, QE)
    wbig = np.concatenate([cross_w[:, :, 0].T, w1, wf[H2:, :]], axis=1)  # [D, 37]
    wb_np = np.ascontiguousarray(
        wbig.reshape(NCHUNK, 128, NW).transpose(1, 0, 2).reshape(128, NCHUNK * NW)
    ).astype(BF16)

    shared = {
        "tblq": tblq,
        "wb": wb_np,
        "w2": w2,
        "wfh": np.ascontiguousarray(wf[:H2, :]),
        "wfh2": np.asarray(np.block([[wf[:H2, :], np.zeros((H2, 1), np.float32)], [np.zeros((H2, 1), np.float32), wf[:H2, :]]]), dtype=np.float32),
        "cb": cross_b.reshape(1, L),
        "b1v": b1.reshape(1, H1),
        "b2v": b2.reshape(1, H2),
        "bfv": bf.reshape(1, 1),
    }

    in_maps = []
    for c in range(NCORES):
        xc = x[c * BC:(c + 1) * BC].astype(np.int64)
        xq = (xc // 4).astype(np.int16)          # [1024, 128]
        xr = (xc % 4).astype(np.int8)
        idx_np = np.empty((NGRP, 128, NIDX // 16), dtype=np.int16)
        msk_np = np.empty((NGRP, 128, 3 * GS), dtype=np.uint8)
        for g in range(NGRP):
            s, hh = g // GPS, g % GPS
            blk = xq[s * 128:(s + 1) * 128, hh * GS:(hh + 1) * GS]  # [128b, GSt]
            lst = blk.T.reshape(-1)                                 # i = t*128+b
            idx_np[g] = np.tile(lst.reshape(NIDX // 16, 16).T, (8, 1))
            rb = xr[s * 128:(s + 1) * 128, hh * GS:(hh + 1) * GS]
            msk_np[g, :, 0 * GS:1 * GS] = (rb == 1).astype(np.uint8)
            msk_np[g, :, 1 * GS:2 * GS] = (rb == 2).astype(np.uint8)
            msk_np[g, :, 2 * GS:3 * GS] = (rb == 3).astype(np.uint8)
        m = dict(shared)
        m["xidx"] = np.ascontiguousarray(idx_np.transpose(1, 0, 2).reshape(128, -1))
        m["xmask"] = np.ascontiguousarray(msk_np.transpose(1, 0, 2).reshape(128, -1))
        in_maps.append(m)
    return in_maps


def _ensure_ntff_hook():
    """The image's antenv lacks axon_hooks; synthesize it so
    run_bass_kernel_spmd(trace=True) can NTFF-profile via the axon .so."""
    import types
    if 'antenv.axon_hooks' in sys.modules:
        return
    import antenv
    mod = types.ModuleType('antenv.axon_hooks')
    _state = {'hook': None}
    def set_axon_ntff_profile_hook(h):
        _state['hook'] = h
    def get_axon_ntff_profile_hook():
        if _state['hook'] is None:
            try:
                from trn_agent_boot.trn_boot import _ntff_profile_via_ctypes
                _state['hook'] = _ntff_profile_via_ctypes('/opt/axon/libaxon_pjrt.so')
            except Exception:
                return None
        return _state['hook']
    mod.set_axon_ntff_profile_hook = set_axon_ntff_profile_hook
    mod.get_axon_ntff_profile_hook = get_axon_ntff_profile_hook
    sys.modules['antenv.axon_hooks'] = mod
    antenv.axon_hooks = mod


def run(inputs: dict, trace: bool = False):
    if trace:
        _ensure_ntff_hook()
    nc = _get_program()
    in_maps = _host_prep(**inputs)
    res = run_bass_kernel_spmd(nc, in_maps, core_ids=list(range(NCORES)),
                               trace=trace)
    out = np.concatenate(
        [np.asarray(res.results[c]["out"]).reshape(BC, 1) for c in range(NCORES)]
    )
    return out.astype(np.float32), res


def kernel(**inputs):
    out, _ = run(inputs, trace=False)
    return out



# revision 10
# speedup vs baseline: 3.1984x; 1.0577x over previous
"""DeepCross kernel for 8x TRN2 NeuronCores.

Math: the cross-network keeps temp = x0 * f with f a per-row scalar, so the
whole model collapses to G = x0 @ [cross_w | w1 | wf_x0]  ([B, 37]) plus a
tiny per-row tail:
    g = G[:, :4]; p1 = G[:, 4:36]; q = G[:, 36:37]
    f1 = 1 + g0 + b0; f2 = f1*g1 + b1; f3 = f2*(1+g2) + b2; f4 = f3*g3 + b3
    h1 = relu(f4 * p1); h2 = relu(h1 @ w2); out = sigmoid(h2 @ wf_h + q + bf)

Device strategy (data-parallel over batch, 1024 rows/core):
  - emb table packed to bf16 row-PAIRS [50000, 256B]; dma_gather with
    idx = x//2 stored int16-WRAPPED (values >= 32768 go negative; the HW
    sign-extends and the address wraps 16.77MB below the in_ap base, so the
    table tensor is 2x sized with the high pairs mirrored into the low half
    and in_ap based at +16.77MB).  One descriptor per lookup, 256B each.
  - gathers run on all 4 SWDGE queues (num_swdge_queues=4) so all 8 GPSIMD
    Q7 cores emit descriptors concurrently (4 pairs).
  - 1-of-2 sub-row select: one base copy + one predicated copy on fp32
    bitcast views (halves DVE/ACT element counts).
  - PE-transpose 128x128 chunks, accumulate G^T [37, 128] per subtile on PE.
  - Tail on 1..37-partition tiles, split in two halves (subs 0-3 issued
    mid-stream for overlap); one [1, 1024] f32 row DMA'd out per core.
"""
import sys
sys.path.insert(0, '/opt/trn_rl_repo')
import os
import numpy as np
import ml_dtypes

from concourse import bass, mybir
import concourse.tile as tile
from concourse import bacc, library_config
from concourse.bass_utils import run_bass_kernel_spmd
from concourse.masks import make_identity
from concourse.tile import add_dep_helper

BF16 = ml_dtypes.bfloat16

B, T, E = 8192, 128, 64
V = 100000
D = T * E                 # 8192
L = 4
H1, H2 = 32, 16
NCORES = 8
BC = B // NCORES          # 1024 batch rows per core
NSUB = BC // 128          # 8 subtiles of 128 rows
GS = 32                   # slots per gather group (quarter subtile)
GPS = 4                   # groups per subtile
NGRP = NSUB * GPS         # 32 gather groups per core
NIDX = 128 * GS           # 4096 real indices per gather
NPAD = 16                 # trailing dummy idxs (defeat trailing-neg trim)
NIDX2 = NIDX + NPAD       # 4112
NBLK = (NIDX2 + 127) // 128   # 33 output row-blocks per gather
PE_EL = 128               # bf16 elements per gathered pair row (256 B)
PAIRS = V // 2            # 50000 pair rows
TBL_N = 131072            # table tensor rows (2x 65536 for the wrap trick)
NCHUNK = D // 128         # 64 d-chunks per subtile
NW = L + H1 + 1           # 37 fused weight columns

_PROGRAM = None
KMODE = os.environ.get('KMODE', 'full')
NSWQ = int(os.environ.get('NSWQ', '4'))     # SWDGE queues (1..4)


def _build_program():
    f32 = mybir.dt.float32
    bf16 = mybir.dt.bfloat16
    nc = bacc.Bacc("TRN2", target_bir_lowering=False, debug=False,
                   num_devices=NCORES, dynamic_dma_scratch_size=32768,
                   num_swdge_queues=NSWQ)

    tblq = nc.dram_tensor("tblq", [TBL_N, PE_EL], bf16, kind="ExternalInput")
    xidx = nc.dram_tensor("xidx", [128, NGRP * (NIDX2 // 16)], mybir.dt.int16,
                          kind="ExternalInput")
    xmask = nc.dram_tensor("xmask", [128, NGRP * GS], mybir.dt.uint8,
                           kind="ExternalInput")
    wbd = nc.dram_tensor("wb", [128, NCHUNK * NW], bf16, kind="ExternalInput")
    w2d = nc.dram_tensor("w2", [H1, H2], f32, kind="ExternalInput")
    wfhd = nc.dram_tensor("wfh", [H2, 1], f32, kind="ExternalInput")
    wfh2d = nc.dram_tensor("wfh2", [H1, 2], f32, kind="ExternalInput")
    cbd = nc.dram_tensor("cb", [1, L], f32, kind="ExternalInput")
    b1d = nc.dram_tensor("b1v", [1, H1], f32, kind="ExternalInput")
    b2d = nc.dram_tensor("b2v", [1, H2], f32, kind="ExternalInput")
    bfd = nc.dram_tensor("bfv", [1, 1], f32, kind="ExternalInput")
    outd = nc.dram_tensor("out", [NSUB, 128], f32, kind="ExternalOutput")

    tbl_ap = tblq.ap()[TBL_N // 2:TBL_N, :]   # base at +16.77MB

    AF = mybir.ActivationFunctionType
    OP = mybir.AluOpType

    with tile.TileContext(nc) as tc:
        with (
            tc.tile_pool(name="const", bufs=1) as cpool,
            tc.tile_pool(name="io", bufs=3) as iopool,
            tc.tile_pool(name="quad", bufs=8) as qpool,
            tc.tile_pool(name="x0c", bufs=4) as xpool,
            tc.tile_pool(name="xt", bufs=4) as xtpool,
            tc.tile_pool(name="tail", bufs=2) as sp,
            tc.tile_pool(name="ptp", bufs=4, space="PSUM") as ptpool,
            tc.tile_pool(name="pgt", bufs=2, space="PSUM") as pgpool,
            tc.tile_pool(name="pts", bufs=2, space="PSUM") as pspool,
        ):
            nc.gpsimd.load_library(library_config.mlp)

            wb_t = cpool.tile([128, NCHUNK * NW], bf16)
            nc.sync.dma_start(out=wb_t[:], in_=wbd.ap())
            ident = cpool.tile([128, 128], bf16)
            make_identity(nc, ident[:])
            ident32 = cpool.tile([128, 128], f32)
            make_identity(nc, ident32[:])
            w2_t = cpool.tile([H1, H2], f32)
            nc.sync.dma_start(out=w2_t[:], in_=w2d.ap())
            wfh_t = cpool.tile([H2, 1], f32)
            nc.sync.dma_start(out=wfh_t[:], in_=wfhd.ap())
            wfh2_t = cpool.tile([H1, 2], f32)
            nc.sync.dma_start(out=wfh2_t[:], in_=wfh2d.ap())
            # pack [cb(4) | b1(32) | b2(16) | bf(1)] into one row, broadcast
            # to all 128 partitions via a k=1 matmul with a ones column.
            NPK = L + H1 + H2 + 1
            pack_t = cpool.tile([1, NPK], f32)
            nc.sync.dma_start(out=pack_t[0:1, 0:L], in_=cbd.ap())
            nc.sync.dma_start(out=pack_t[0:1, L:L + H1], in_=b1d.ap())
            nc.sync.dma_start(out=pack_t[0:1, L + H1:L + H1 + H2], in_=b2d.ap())
            nc.sync.dma_start(out=pack_t[0:1, L + H1 + H2:NPK], in_=bfd.ap())
            ones_r = cpool.tile([1, 128], f32)
            nc.vector.memset(ones_r[:], 1.0)
            packb_p = pspool.tile([128, NPK], f32, tag="tps")
            packb_mm = nc.tensor.matmul(out=packb_p[:], lhsT=ones_r[:],
                                        rhs=pack_t[:], start=True, stop=True)
            prev_tail_pe0 = packb_mm.ins
            packb = cpool.tile([128, NPK], f32)
            nc.vector.tensor_copy(out=packb[:], in_=packb_p[:])
            cbb = packb[:, 0:L]
            b1b = packb[:, L:L + H1]
            b2b = packb[:, L + H1:L + H1 + H2]
            bfb = packb[:, L + H1 + H2:NPK]
            out_col = None
            if KMODE != "notail":
                out_col = cpool.tile([128, NSUB], f32, tag="out_col")
            gts_all = cpool.tile([NW, BC], f32, tag="gts_all")
            idx_all = cpool.tile([128, NGRP * (NIDX2 // 16)], mybir.dt.int16,
                                 tag="idx_all")
            nc.sync.dma_start(out=idx_all[:], in_=xidx.ap())
            msk_all = cpool.tile([128, NGRP * GS], mybir.dt.uint8,
                                 tag="msk_all")
            nc.sync.dma_start(out=msk_all[:], in_=xmask.ap())

            gtt_all = None
            if KMODE != "notail":
                gtt_all = cpool.tile([128, NSUB * NW], f32, tag="gtt_all")

            def tail_half(s0):
                """Tail math for subtiles [s0, s0+4): f-recurrence, deep MLP,
                sigmoid into out_col[:, s0:s0+4]."""
                ns = 4
                for s in range(s0, s0 + ns):
                    gtt_p = pspool.tile([128, 64], f32, tag="tps")
                    nc.tensor.transpose(
                        out=gtt_p[:, 0:NW],
                        in_=gts_all[:, s * 128:(s + 1) * 128],
                        identity=ident32[0:NW, 0:NW])
                    nc.scalar.copy(out=gtt_all[:, s * NW:(s + 1) * NW],
                                   in_=gtt_p[:, 0:NW])
                gv = gtt_all[:, s0 * NW:(s0 + ns) * NW].rearrange(
                    "p (s w) -> p s w", w=NW)

                f1 = cpool.tile([128, ns], f32, tag="f1")
                nc.vector.tensor_scalar(out=f1[:], in0=gv[:, :, 0:1],
                                        scalar1=cbb[:, 0:1], scalar2=1.0,
                                        op0=OP.add, op1=OP.add)
                f2 = cpool.tile([128, ns], f32, tag="f2")
                nc.vector.tensor_tensor(out=f2[:], in0=f1[:].rearrange(
                    "p (s one) -> p s one", one=1), in1=gv[:, :, 1:2],
                    op=OP.mult)
                nc.vector.tensor_scalar(out=f2[:], in0=f2[:],
                                        scalar1=cbb[:, 1:2], scalar2=None,
                                        op0=OP.add)
                u3 = cpool.tile([128, ns], f32, tag="u3")
                nc.vector.tensor_scalar(out=u3[:], in0=gv[:, :, 2:3],
                                        scalar1=1.0, scalar2=None, op0=OP.add)
                f3 = cpool.tile([128, ns], f32, tag="f3")
                nc.vector.tensor_tensor(out=f3[:], in0=f2[:], in1=u3[:],
                                        op=OP.mult)
                nc.vector.tensor_scalar(out=f3[:], in0=f3[:],
                                        scalar1=cbb[:, 2:3], scalar2=None,
                                        op0=OP.add)
                f4 = cpool.tile([128, ns], f32, tag="f4")
                nc.vector.tensor_tensor(out=f4[:], in0=f3[:].rearrange(
                    "p (s one) -> p s one", one=1), in1=gv[:, :, 3:4],
                    op=OP.mult)
                nc.vector.tensor_scalar(out=f4[:], in0=f4[:],
                                        scalar1=cbb[:, 3:4], scalar2=None,
                                        op0=OP.add)

                # h1 = relu(f4 * p1 + b1)  [128, ns, H1]
                h1_all = cpool.tile([128, ns * H1], f32, tag="h1_all")
                h1v = h1_all[:].rearrange("p (s h) -> p s h", h=H1)
                nc.vector.tensor_tensor(
                    out=h1v, in0=gv[:, :, L:L + H1],
                    in1=f4[:].rearrange("p (s one) -> p s one", one=1)
                        .to_broadcast([128, ns, H1]),
                    op=OP.mult)
                nc.vector.tensor_tensor(
                    out=h1v, in0=h1v,
                    in1=b1b.rearrange("p (one h) -> p one h", one=1)
                        .to_broadcast([128, ns, H1]),
                    op=OP.add)
                nc.scalar.activation(out=h1_all[:], in_=h1_all[:], func=AF.Relu)

                # h2 = relu(h1 @ w2 + b2): 4 subs pack one [128,128] transpose
                h2_all = cpool.tile([128, ns * H2], f32, tag="h2_all")
                h1T_p = pspool.tile([128, 128], f32, tag="tps")
                nc.tensor.transpose(out=h1T_p[:], in_=h1_all[:],
                                    identity=ident32[:])
                h1T = cpool.tile([128, 128], f32, tag="h1T")
                nc.scalar.copy(out=h1T[:], in_=h1T_p[:])
                for j in range(ns):
                    h1Tj = cpool.tile([H1, 128], f32, tag="h1Tj")
                    nc.scalar.copy(out=h1Tj[:],
                                   in_=h1T[j * H1:(j + 1) * H1, :])
                    h2p = pspool.tile([128, H2], f32, tag="tps")
                    nc.tensor.matmul(out=h2p[:], lhsT=h1Tj[:],
                                     rhs=w2_t[:], start=True, stop=True)
                    nc.scalar.copy(out=h2_all[:, j * H2:(j + 1) * H2],
                                   in_=h2p[:])
                h2v = h2_all[:].rearrange("p (s h) -> p s h", h=H2)
                nc.vector.tensor_tensor(
                    out=h2v, in0=h2v,
                    in1=b2b.rearrange("p (one h) -> p one h", one=1)
                        .to_broadcast([128, ns, H2]),
                    op=OP.add)
                nc.scalar.activation(out=h2_all[:], in_=h2_all[:], func=AF.Relu)

                # z = h2 @ wf_h ; out = sigmoid(z + q + bf)
                z_all = cpool.tile([128, ns], f32, tag="z_all")
                h2T_p = pspool.tile([128, 128], f32, tag="tps")
                nc.tensor.transpose(out=h2T_p[0:ns * H2, :], in_=h2_all[:],
                                    identity=ident32[:])
                h2T = cpool.tile([ns * H2, 128], f32, tag="h2T")
                nc.scalar.copy(out=h2T[:], in_=h2T_p[0:ns * H2, :])
                for pr in range(ns // 2):
                    h2Tp2 = cpool.tile([2 * H2, 128], f32, tag="h2Tp2")
                    nc.scalar.copy(out=h2Tp2[:],
                                   in_=h2T[pr * 2 * H2:(pr + 1) * 2 * H2, :])
                    zp = pspool.tile([128, 2], f32, tag="tps")
                    nc.tensor.matmul(out=zp[:], lhsT=h2Tp2[:],
                                     rhs=wfh2_t[:], start=True, stop=True)
                    nc.scalar.copy(out=z_all[:, pr * 2:(pr + 1) * 2],
                                   in_=zp[:])
                nc.vector.tensor_tensor(out=z_all[:], in0=z_all[:].rearrange(
                    "p (s one) -> p s one", one=1), in1=gv[:, :, NW - 1:NW],
                    op=OP.add)
                nc.scalar.activation(out=out_col[:, s0:s0 + ns], in_=z_all[:],
                                     func=AF.Sigmoid,
                                     bias=bfb[:, 0:1], scale=1.0)

            prev_tail_pe = prev_tail_pe0
            for sub in range(NSUB):
                gt = pgpool.tile([NW, 128], f32, tag="gt")
                for h in range(GPS):
                    g = sub * GPS + h
                    idx_t = idx_all[:, g * (NIDX2 // 16):(g + 1) * (NIDX2 // 16)]
                    msk_t = msk_all[:, g * GS:(g + 1) * GS]

                    quad = qpool.tile([128, NBLK * PE_EL], bf16, tag="quad")
                    qview = quad[:].rearrange("p (s e) -> p s e", e=PE_EL)
                    nc.gpsimd.dma_gather(
                        out_ap=qview[:, :, :],
                        in_ap=tbl_ap,
                        idxs_ap=idx_t[:],
                        num_idxs=NIDX2,
                        num_idxs_reg=NIDX2,
                        elem_size=PE_EL,
                        single_packet=False,
                        queue_num=g % NSWQ,
                    )

                    x0c = xpool.tile([128, GS * 64], bf16, tag="x0c")
                    qv32 = quad[:].bitcast(f32).rearrange("p (s e) -> p s e",
                                                          e=PE_EL // 2)
                    xv32 = x0c[:].bitcast(f32).rearrange("p (s e) -> p s e",
                                                         e=32)
                    HG = GS // 2

                    def _mk(lo, hi):
                        m = msk_t[:, lo:hi]
                        m = m.rearrange("p (s one) -> p s one", one=1)
                        return m.to_broadcast([128, hi - lo, 32])

                    for lo in (0, HG):
                        hi = lo + HG
                        nc.scalar.copy(out=xv32[:, lo:hi, :],
                                       in_=qv32[:, lo:hi, 0:32])
                        nc.vector.copy_predicated(out=xv32[:, lo:hi, :],
                                                  mask=_mk(lo, hi),
                                                  data=qv32[:, lo:hi, 32:64])
                    for c4 in range(GS // 8):   # groups of 4 chunks
                        tp = ptpool.tile([128, 512], bf16, tag="tp")
                        for j in range(4):
                            c2 = c4 * 4 + j
                            nc.tensor.transpose(
                                out=tp[:, j * 128:(j + 1) * 128],
                                in_=x0c[:, c2 * 128:(c2 + 1) * 128],
                                identity=ident[:],
                            )
                        xt = xtpool.tile([128, 512], bf16, tag="xt")
                        nc.scalar.copy(out=xt[:], in_=tp[:])
                        for j in range(4):
                            cd = h * (GS // 2) + c4 * 4 + j
                            mm = nc.tensor.matmul(
                                out=gt[:],
                                lhsT=wb_t[:, cd * NW:(cd + 1) * NW],
                                rhs=xt[:, j * 128:(j + 1) * 128],
                                start=(cd == 0),
                                stop=(cd == NCHUNK - 1),
                            )
                            if cd == 0 and sub == 0 and prev_tail_pe is not None:
                                add_dep_helper(mm.ins, prev_tail_pe,
                                               reason="packb before accum groups")

                # stash G^T for the batched tail
                nc.scalar.copy(out=gts_all[:, sub * 128:(sub + 1) * 128],
                               in_=gt[:])
                if KMODE == "notail" and sub == 0:
                    nc.sync.dma_start(out=outd.ap(), in_=gts_all[0:NSUB, 0:128])
                if KMODE != "notail" and sub == 4:
                    tail_half(0)    # overlap subs 0-3 tail with sub 5-7 work

            if KMODE != "notail":
                tail_half(4)
                oT_p = pspool.tile([NSUB, 128], f32, tag="tps")
                nc.tensor.transpose(out=oT_p[:], in_=out_col[:],
                                    identity=ident32[:])
                oT = cpool.tile([NSUB, 128], f32)
                nc.scalar.copy(out=oT[:], in_=oT_p[:])
                nc.sync.dma_start(out=outd.ap(), in_=oT[:])

    nc.compile()
    return nc


def _get_program():
    global _PROGRAM
    if _PROGRAM is None:
        _PROGRAM = _build_program()
    return _PROGRAM


def _host_prep(x, emb, cross_w, cross_b, w1, b1, w2, b2, wf, bf):
    x = np.asarray(x)
    emb = np.ascontiguousarray(np.asarray(emb, dtype=np.float32))
    cross_w = np.asarray(cross_w, dtype=np.float32)
    cross_b = np.asarray(cross_b, dtype=np.float32)
    w1 = np.asarray(w1, dtype=np.float32)
    w2 = np.asarray(w2, dtype=np.float32)
    b1 = np.asarray(b1, dtype=np.float32)
    b2 = np.asarray(b2, dtype=np.float32)
    wf = np.asarray(wf, dtype=np.float32)
    bf = np.asarray(bf, dtype=np.float32)

    # pair table with the int16-wrap mirror: real pairs at [65536, 65536+50000),
    # pairs >= 32768 mirrored at their raw index for wrapped (negative) idxs.
    pe = emb.astype(BF16).reshape(PAIRS, PE_EL)
    tbl = np.zeros((TBL_N, PE_EL), dtype=BF16)
    tbl[TBL_N // 2:TBL_N // 2 + PAIRS] = pe
    tbl[32768:PAIRS] = pe[32768:PAIRS]

    wbig = np.concatenate([cross_w[:, :, 0].T, w1, wf[H2:, :]], axis=1)  # [D, 37]
    wb_np = np.ascontiguousarray(
        wbig.reshape(NCHUNK, 128, NW).transpose(1, 0, 2).reshape(128, NCHUNK * NW)
    ).astype(BF16)

    shared = {
        "tblq": tbl,
        "wb": wb_np,
        "w2": w2,
        "wfh": np.ascontiguousarray(wf[:H2, :]),
        "wfh2": np.asarray(np.block([[wf[:H2, :], np.zeros((H2, 1), np.float32)], [np.zeros((H2, 1), np.float32), wf[:H2, :]]]), dtype=np.float32),
        "cb": cross_b.reshape(1, L),
        "b1v": b1.reshape(1, H1),
        "b2v": b2.reshape(1, H2),
        "bfv": bf.reshape(1, 1),
    }

    in_maps = []
    for c in range(NCORES):
        xc = x[c * BC:(c + 1) * BC].astype(np.int64)
        xq = (xc // 2).astype(np.int32)           # pair idx, wraps to int16
        xr = (xc % 2).astype(np.uint8)
        idx_np = np.empty((NGRP, 128, NIDX2 // 16), dtype=np.int16)
        msk_np = np.empty((NGRP, 128, GS), dtype=np.uint8)
        for g in range(NGRP):
            s, hh = g // GPS, g % GPS
            blk = xq[s * 128:(s + 1) * 128, hh * GS:(hh + 1) * GS]  # [128b, GSt]
            lst = np.concatenate([blk.T.reshape(-1),                # i = t*128+b
                                  np.zeros(NPAD, dtype=np.int32)])
            idx_np[g] = np.tile(
                lst.reshape(NIDX2 // 16, 16).T.astype(np.int16), (8, 1))
            msk_np[g] = xr[s * 128:(s + 1) * 128, hh * GS:(hh + 1) * GS]
        m = dict(shared)
        m["xidx"] = np.ascontiguousarray(idx_np.transpose(1, 0, 2).reshape(128, -1))
        m["xmask"] = np.ascontiguousarray(msk_np.transpose(1, 0, 2).reshape(128, -1))
        in_maps.append(m)
    return in_maps


def _ensure_ntff_hook():
    """The image's antenv lacks axon_hooks; synthesize it so
    run_bass_kernel_spmd(trace=True) can NTFF-profile via the axon .so."""
    import types
    if 'antenv.axon_hooks' in sys.modules:
        return
    import antenv
    mod = types.ModuleType('antenv.axon_hooks')
    _state = {'hook': None}
    def set_axon_ntff_profile_hook(h):
        _state['hook'] = h
    def get_axon_ntff_profile_hook():
        if _state['hook'] is None:
            try:
                from trn_agent_boot.trn_boot import _ntff_profile_via_ctypes
                _state['hook'] = _ntff_profile_via_ctypes('/opt/axon/libaxon_pjrt.so')
            except Exception:
                return None
        return _state['hook']
    mod.set_axon_ntff_profile_hook = set_axon_ntff_profile_hook
    mod.get_axon_ntff_profile_hook = get_axon_ntff_profile_hook
    sys.modules['antenv.axon_hooks'] = mod
    antenv.axon_hooks = mod


def run(inputs: dict, trace: bool = False):
    if trace:
        _ensure_ntff_hook()
    nc = _get_program()
    in_maps = _host_prep(**inputs)
    res = run_bass_kernel_spmd(nc, in_maps, core_ids=list(range(NCORES)),
                               trace=trace)
    out = np.concatenate(
        [np.asarray(res.results[c]["out"]).reshape(BC, 1) for c in range(NCORES)]
    )
    return out.astype(np.float32), res


def kernel(**inputs):
    out, _ = run(inputs, trace=False)
    return out


# revision 17
# speedup vs baseline: 3.2810x; 1.0258x over previous
"""DeepCross kernel for 8x TRN2 NeuronCores.

Math: the cross-network keeps temp = x0 * f with f a per-row scalar, so the
whole model collapses to G = x0 @ [cross_w | w1 | wf_x0]  ([B, 37]) plus a
tiny per-row tail:
    g = G[:, :4]; p1 = G[:, 4:36]; q = G[:, 36:37]
    f1 = 1 + g0 + b0; f2 = f1*g1 + b1; f3 = f2*(1+g2) + b2; f4 = f3*g3 + b3
    h1 = relu(f4 * p1); h2 = relu(h1 @ w2); out = sigmoid(h2 @ wf_h + q + bf)

Device strategy (data-parallel over batch, 1024 rows/core):
  - emb table packed to bf16 row-PAIRS [50000, 256B]; dma_gather with
    idx = x//2 stored int16-WRAPPED (values >= 32768 go negative; the HW
    sign-extends and the address wraps 16.77MB below the in_ap base, so the
    table tensor is 2x sized with the high pairs mirrored into the low half
    and in_ap based at +16.77MB).  One descriptor per lookup, 256B each.
  - gathers run on all 4 SWDGE queues (num_swdge_queues=4) so all 8 GPSIMD
    Q7 cores emit descriptors concurrently (4 pairs).
  - 1-of-2 sub-row select: one base copy + one predicated copy on fp32
    bitcast views (halves DVE/ACT element counts).
  - PE-transpose 128x128 chunks, accumulate G^T [37, 128] per subtile on PE.
  - Tail on 1..37-partition tiles, split in two halves (subs 0-3 issued
    mid-stream for overlap); one [1, 1024] f32 row DMA'd out per core.
"""
import sys
sys.path.insert(0, '/opt/trn_rl_repo')
import os
import numpy as np
import ml_dtypes

from concourse import bass, mybir
import concourse.tile as tile
from concourse import bacc, library_config
from concourse.bass_utils import run_bass_kernel_spmd
from concourse.masks import make_identity
from concourse.tile import add_dep_helper

BF16 = ml_dtypes.bfloat16

B, T, E = 8192, 128, 64
V = 100000
D = T * E                 # 8192
L = 4
H1, H2 = 32, 16
NCORES = 8
BC = B // NCORES          # 1024 batch rows per core
NSUB = BC // 128          # 8 subtiles of 128 rows
GS = 32                   # slots per gather group (quarter subtile)
GPS = 4                   # groups per subtile
NGRP = NSUB * GPS         # 32 gather groups per core
NIDX = 128 * GS           # 4096 real indices per gather
NPAD = 16                 # trailing dummy idxs (defeat trailing-neg trim)
NIDX2 = NIDX + NPAD       # 4112
NBLK = (NIDX2 + 127) // 128   # 33 output row-blocks per gather
PE_EL = 128               # bf16 elements per gathered pair row (256 B)
PAIRS = V // 2            # 50000 pair rows
TBL_N = 131072            # table tensor rows (2x 65536 for the wrap trick)
NCHUNK = D // 128         # 64 d-chunks per subtile
NW = L + H1 + 1           # 37 fused weight columns

_PROGRAM = None
KMODE = os.environ.get('KMODE', 'full')
NSWQ = int(os.environ.get('NSWQ', '4'))     # SWDGE queues (1..4)


def _build_program():
    f32 = mybir.dt.float32
    bf16 = mybir.dt.bfloat16
    nc = bacc.Bacc("TRN2", target_bir_lowering=False, debug=False,
                   num_devices=NCORES, dynamic_dma_scratch_size=32768,
                   num_swdge_queues=NSWQ)

    tblq = nc.dram_tensor("tblq", [TBL_N, PE_EL], bf16, kind="ExternalInput")
    xidx = nc.dram_tensor("xidx", [128, NGRP * (NIDX2 // 16)], mybir.dt.int16,
                          kind="ExternalInput")
    xmask = nc.dram_tensor("xmask", [128, NGRP * GS], mybir.dt.uint8,
                           kind="ExternalInput")
    wbd = nc.dram_tensor("wb", [128, NCHUNK * NW], bf16, kind="ExternalInput")
    # packed tail constants: [cb(4) | b1(32) | b2(16) | bf(1) | w2T(512) | wfh(16)]
    NPK = L + H1 + H2 + 1 + H1 * H2 + H2
    packd = nc.dram_tensor("packv", [1, NPK], f32, kind="ExternalInput")
    outd = nc.dram_tensor("out", [NSUB, 128], f32, kind="ExternalOutput")

    tbl_ap = tblq.ap()[TBL_N // 2:TBL_N, :]   # base at +16.77MB

    AF = mybir.ActivationFunctionType
    OP = mybir.AluOpType

    with tile.TileContext(nc) as tc:
        with (
            tc.tile_pool(name="const", bufs=1) as cpool,
            tc.tile_pool(name="io", bufs=3) as iopool,
            tc.tile_pool(name="quad", bufs=8) as qpool,
            tc.tile_pool(name="x0c", bufs=4) as xpool,
            tc.tile_pool(name="xt", bufs=4) as xtpool,
            tc.tile_pool(name="tail", bufs=2) as sp,
            tc.tile_pool(name="ptp", bufs=4, space="PSUM") as ptpool,
            tc.tile_pool(name="pgt", bufs=2, space="PSUM") as pgpool,
            tc.tile_pool(name="pts", bufs=2, space="PSUM") as pspool,
        ):
            nc.gpsimd.load_library(library_config.mlp)

            # idx upload split so wave-0 gathers start as soon as possible
            NIW = NIDX2 // 16
            idx_all = cpool.tile([128, NGRP * NIW], mybir.dt.int16,
                                 tag="idx_all")
            nc.sync.dma_start(out=idx_all[:, 0:4 * NIW],
                              in_=xidx.ap()[:, 0:4 * NIW])
            nc.sync.dma_start(out=idx_all[:, 4 * NIW:],
                              in_=xidx.ap()[:, 4 * NIW:])
            msk_all = cpool.tile([128, NGRP * GS], mybir.dt.uint8,
                                 tag="msk_all")
            nc.sync.dma_start(out=msk_all[:], in_=xmask.ap())

            wb_t = cpool.tile([128, NCHUNK * NW], bf16)
            nc.sync.dma_start(out=wb_t[:], in_=wbd.ap())
            ident = cpool.tile([128, 128], bf16)
            make_identity(nc, ident[:])
            ident32 = cpool.tile([128, 128], f32)
            make_identity(nc, ident32[:])
            # broadcast packed tail constants to all 128 partitions via a
            # k=1 matmul with a ones column.
            pack_t = cpool.tile([1, NPK], f32)
            nc.sync.dma_start(out=pack_t[:], in_=packd.ap())
            ones_r = cpool.tile([1, 128], f32)
            nc.vector.memset(ones_r[:], 1.0)
            packb = cpool.tile([128, NPK], f32)
            packb_mm = None
            for off in range(0, NPK, 512):
                w = min(512, NPK - off)
                pb_p = pspool.tile([128, 512], f32, tag="tps")
                packb_mm = nc.tensor.matmul(out=pb_p[:, 0:w], lhsT=ones_r[:],
                                            rhs=pack_t[0:1, off:off + w],
                                            start=True, stop=True)
                nc.vector.tensor_copy(out=packb[:, off:off + w],
                                      in_=pb_p[:, 0:w])
            prev_tail_pe0 = packb_mm.ins
            cbb = packb[:, 0:L]
            b1b = packb[:, L:L + H1]
            b2b = packb[:, L + H1:L + H1 + H2]
            bfb = packb[:, L + H1 + H2:L + H1 + H2 + 1]
            OW2 = L + H1 + H2 + 1
            w2b = packb[:, OW2:OW2 + H1 * H2]        # w2T j-major [16, 32]
            wfhb = packb[:, OW2 + H1 * H2:NPK]       # [16]
            out_col = None
            if KMODE != "notail":
                out_col = cpool.tile([128, NSUB], f32, tag="out_col")
            gts_all = cpool.tile([NW, BC], f32, tag="gts_all")

            gtt_all = None
            if KMODE != "notail":
                gtt_all = cpool.tile([128, NSUB * NW], f32, tag="gtt_all")

            def tail_half(s0, ns):
                """Tail math for subtiles [s0, s0+ns): f-recurrence, deep MLP
                done as DVE multiply+reduce, sigmoid into out_col[:, s0:s0+ns].
                """
                for s in range(s0, s0 + ns):
                    gtt_p = pspool.tile([128, 64], f32, tag="tps")
                    nc.tensor.transpose(
                        out=gtt_p[:, 0:NW],
                        in_=gts_all[:, s * 128:(s + 1) * 128],
                        identity=ident32[0:NW, 0:NW])
                    nc.scalar.copy(out=gtt_all[:, s * NW:(s + 1) * NW],
                                   in_=gtt_p[:, 0:NW])
                gv = gtt_all[:, s0 * NW:(s0 + ns) * NW].rearrange(
                    "p (s w) -> p s w", w=NW)

                f1 = cpool.tile([128, ns], f32, tag=f"f1_{s0}")
                nc.vector.tensor_scalar(out=f1[:], in0=gv[:, :, 0:1],
                                        scalar1=cbb[:, 0:1], scalar2=1.0,
                                        op0=OP.add, op1=OP.add)
                f2 = cpool.tile([128, ns], f32, tag=f"f2_{s0}")
                nc.vector.tensor_tensor(out=f2[:], in0=f1[:].rearrange(
                    "p (s one) -> p s one", one=1), in1=gv[:, :, 1:2],
                    op=OP.mult)
                nc.vector.tensor_scalar(out=f2[:], in0=f2[:],
                                        scalar1=cbb[:, 1:2], scalar2=None,
                                        op0=OP.add)
                u3 = cpool.tile([128, ns], f32, tag=f"u3_{s0}")
                nc.vector.tensor_scalar(out=u3[:], in0=gv[:, :, 2:3],
                                        scalar1=1.0, scalar2=None, op0=OP.add)
                f3 = cpool.tile([128, ns], f32, tag=f"f3_{s0}")
                nc.vector.tensor_tensor(out=f3[:], in0=f2[:], in1=u3[:],
                                        op=OP.mult)
                nc.vector.tensor_scalar(out=f3[:], in0=f3[:],
                                        scalar1=cbb[:, 2:3], scalar2=None,
                                        op0=OP.add)
                f4 = cpool.tile([128, ns], f32, tag=f"f4_{s0}")
                nc.vector.tensor_tensor(out=f4[:], in0=f3[:].rearrange(
                    "p (s one) -> p s one", one=1), in1=gv[:, :, 3:4],
                    op=OP.mult)
                nc.vector.tensor_scalar(out=f4[:], in0=f4[:],
                                        scalar1=cbb[:, 3:4], scalar2=None,
                                        op0=OP.add)

                # h1 = relu(f4 * p1 + b1)  [128, ns, H1]
                h1_all = cpool.tile([128, ns * H1], f32, tag=f"h1_{s0}")
                h1v = h1_all[:].rearrange("p (s h) -> p s h", h=H1)
                nc.vector.tensor_tensor(
                    out=h1v, in0=gv[:, :, L:L + H1],
                    in1=f4[:].rearrange("p (s one) -> p s one", one=1)
                        .to_broadcast([128, ns, H1]),
                    op=OP.mult)
                nc.vector.tensor_tensor(
                    out=h1v, in0=h1v,
                    in1=b1b.rearrange("p (one h) -> p one h", one=1)
                        .to_broadcast([128, ns, H1]),
                    op=OP.add)
                nc.scalar.activation(out=h1_all[:], in_=h1_all[:], func=AF.Relu)

                # h2 = relu(h1 @ w2 + b2) via DVE: mult vs w2T then reduce
                ht = cpool.tile([128, ns * H2 * H1], f32, tag=f"ht_{s0}")
                htv = ht[:].rearrange("p (s j e) -> p s j e", j=H2, e=H1)
                nc.vector.tensor_tensor(
                    out=htv,
                    in0=h1v.rearrange("p s (one e) -> p s one e", one=1)
                        .to_broadcast([128, ns, H2, H1]),
                    in1=w2b.rearrange("p (one j e) -> p one j e", one=1, j=H2)
                        .to_broadcast([128, ns, H2, H1]),
                    op=OP.mult)
                h2_all = cpool.tile([128, ns * H2], f32, tag=f"h2_{s0}")
                h2v = h2_all[:].rearrange("p (s j) -> p s j", j=H2)
                nc.vector.tensor_reduce(
                    out=h2v.rearrange("p s (j one) -> p s j one", one=1),
                    in_=htv, axis=mybir.AxisListType.X, op=OP.add)
                nc.vector.tensor_tensor(
                    out=h2v, in0=h2v,
                    in1=b2b.rearrange("p (one h) -> p one h", one=1)
                        .to_broadcast([128, ns, H2]),
                    op=OP.add)
                nc.scalar.activation(out=h2_all[:], in_=h2_all[:], func=AF.Relu)

                # z = h2 @ wf_h ; out = sigmoid(z + q + bf)
                zt = cpool.tile([128, ns * H2], f32, tag=f"zt_{s0}")
                ztv = zt[:].rearrange("p (s j) -> p s j", j=H2)
                nc.vector.tensor_tensor(
                    out=ztv, in0=h2v,
                    in1=wfhb.rearrange("p (one j) -> p one j", one=1)
                        .to_broadcast([128, ns, H2]),
                    op=OP.mult)
                z_all = cpool.tile([128, ns], f32, tag=f"z_{s0}")
                nc.vector.tensor_reduce(
                    out=z_all[:].rearrange("p (s one) -> p s one", one=1),
                    in_=ztv, axis=mybir.AxisListType.X, op=OP.add)
                nc.vector.tensor_tensor(out=z_all[:], in0=z_all[:].rearrange(
                    "p (s one) -> p s one", one=1), in1=gv[:, :, NW - 1:NW],
                    op=OP.add)
                nc.scalar.activation(out=out_col[:, s0:s0 + ns], in_=z_all[:],
                                     func=AF.Sigmoid,
                                     bias=bfb[:, 0:1], scale=1.0)

            prev_tail_pe = prev_tail_pe0
            for sub in range(NSUB):
                gt = pgpool.tile([NW, 128], f32, tag="gt")
                for h in range(GPS):
                    g = sub * GPS + h
                    idx_t = idx_all[:, g * (NIDX2 // 16):(g + 1) * (NIDX2 // 16)]
                    msk_t = msk_all[:, g * GS:(g + 1) * GS]

                    quad = qpool.tile([128, NBLK * PE_EL], bf16, tag="quad")
                    qview = quad[:].rearrange("p (s e) -> p s e", e=PE_EL)
                    nc.gpsimd.dma_gather(
                        out_ap=qview[:, :, :],
                        in_ap=tbl_ap,
                        idxs_ap=idx_t[:],
                        num_idxs=NIDX2,
                        num_idxs_reg=NIDX2,
                        elem_size=PE_EL,
                        single_packet=False,
                        queue_num=g % NSWQ,
                    )

                    x0c = xpool.tile([128, GS * 64], bf16, tag="x0c")
                    qv32 = quad[:].bitcast(f32).rearrange("p (s e) -> p s e",
                                                          e=PE_EL // 2)
                    xv32 = x0c[:].bitcast(f32).rearrange("p (s e) -> p s e",
                                                         e=32)
                    HG = GS // 2

                    def _mk(lo, hi):
                        m = msk_t[:, lo:hi]
                        m = m.rearrange("p (s one) -> p s one", one=1)
                        return m.to_broadcast([128, hi - lo, 32])

                    for lo in (0, HG):
                        hi = lo + HG
                        nc.scalar.copy(out=xv32[:, lo:hi, :],
                                       in_=qv32[:, lo:hi, 0:32])
                        nc.vector.copy_predicated(out=xv32[:, lo:hi, :],
                                                  mask=_mk(lo, hi),
                                                  data=qv32[:, lo:hi, 32:64])
                    for c4 in range(GS // 8):   # groups of 4 chunks
                        tp = ptpool.tile([128, 512], bf16, tag="tp")
                        for j in range(4):
                            c2 = c4 * 4 + j
                            nc.tensor.transpose(
                                out=tp[:, j * 128:(j + 1) * 128],
                                in_=x0c[:, c2 * 128:(c2 + 1) * 128],
                                identity=ident[:],
                            )
                        xt = xtpool.tile([128, 512], bf16, tag="xt")
                        nc.scalar.copy(out=xt[:], in_=tp[:])
                        for j in range(4):
                            cd = h * (GS // 2) + c4 * 4 + j
                            mm = nc.tensor.matmul(
                                out=gt[:],
                                lhsT=wb_t[:, cd * NW:(cd + 1) * NW],
                                rhs=xt[:, j * 128:(j + 1) * 128],
                                start=(cd == 0),
                                stop=(cd == NCHUNK - 1),
                            )
                            if cd == 0 and sub == 0 and prev_tail_pe is not None:
                                add_dep_helper(mm.ins, prev_tail_pe,
                                               reason="packb before accum groups")

                # stash G^T for the batched tail
                nc.scalar.copy(out=gts_all[:, sub * 128:(sub + 1) * 128],
                               in_=gt[:])
                if KMODE == "notail" and sub == 0:
                    nc.sync.dma_start(out=outd.ap(), in_=gts_all[0:NSUB, 0:128])
                if KMODE != "notail" and sub == 4:
                    tail_half(0, 4)   # overlap subs 0-3 tail with sub 5-7 work
                if KMODE != "notail" and sub == 6:
                    tail_half(4, 2)   # subs 4-5 while sub 7 streams

            if KMODE != "notail":
                tail_half(6, 2)
                oT_p = pspool.tile([NSUB, 128], f32, tag="tps")
                nc.tensor.transpose(out=oT_p[:], in_=out_col[:],
                                    identity=ident32[:])
                oT = cpool.tile([NSUB, 128], f32)
                nc.scalar.copy(out=oT[:], in_=oT_p[:])
                nc.sync.dma_start(out=outd.ap(), in_=oT[:])

    nc.compile()
    return nc


def _get_program():
    global _PROGRAM
    if _PROGRAM is None:
        _PROGRAM = _build_program()
    return _PROGRAM


def _host_prep(x, emb, cross_w, cross_b, w1, b1, w2, b2, wf, bf):
    x = np.asarray(x)
    emb = np.ascontiguousarray(np.asarray(emb, dtype=np.float32))
    cross_w = np.asarray(cross_w, dtype=np.float32)
    cross_b = np.asarray(cross_b, dtype=np.float32)
    w1 = np.asarray(w1, dtype=np.float32)
    w2 = np.asarray(w2, dtype=np.float32)
    b1 = np.asarray(b1, dtype=np.float32)
    b2 = np.asarray(b2, dtype=np.float32)
    wf = np.asarray(wf, dtype=np.float32)
    bf = np.asarray(bf, dtype=np.float32)

    # pair table with the int16-wrap mirror: real pairs at [65536, 65536+50000),
    # pairs >= 32768 mirrored at their raw index for wrapped (negative) idxs.
    pe = emb.astype(BF16).reshape(PAIRS, PE_EL)
    tbl = np.zeros((TBL_N, PE_EL), dtype=BF16)
    tbl[TBL_N // 2:TBL_N // 2 + PAIRS] = pe
    tbl[32768:PAIRS] = pe[32768:PAIRS]

    wbig = np.concatenate([cross_w[:, :, 0].T, w1, wf[H2:, :]], axis=1)  # [D, 37]
    wb_np = np.ascontiguousarray(
        wbig.reshape(NCHUNK, 128, NW).transpose(1, 0, 2).reshape(128, NCHUNK * NW)
    ).astype(BF16)

    # [cb(4) | b1(32) | b2(16) | bf(1) | w2T j-major (512) | wfh (16)]
    packv = np.concatenate([
        cross_b.reshape(-1), b1.reshape(-1), b2.reshape(-1), bf.reshape(-1),
        w2.T.reshape(-1), wf[:H2, 0].reshape(-1),
    ]).astype(np.float32).reshape(1, -1)

    shared = {
        "tblq": tbl,
        "wb": wb_np,
        "packv": packv,
    }

    in_maps = []
    for c in range(NCORES):
        xc = x[c * BC:(c + 1) * BC].astype(np.int64)
        xq = (xc // 2).astype(np.int32)           # pair idx, wraps to int16
        xr = (xc % 2).astype(np.uint8)
        idx_np = np.empty((NGRP, 128, NIDX2 // 16), dtype=np.int16)
        msk_np = np.empty((NGRP, 128, GS), dtype=np.uint8)
        for g in range(NGRP):
            s, hh = g // GPS, g % GPS
            blk = xq[s * 128:(s + 1) * 128, hh * GS:(hh + 1) * GS]  # [128b, GSt]
            lst = np.concatenate([blk.T.reshape(-1),                # i = t*128+b
                                  np.zeros(NPAD, dtype=np.int32)])
            idx_np[g] = np.tile(
                lst.reshape(NIDX2 // 16, 16).T.astype(np.int16), (8, 1))
            msk_np[g] = xr[s * 128:(s + 1) * 128, hh * GS:(hh + 1) * GS]
        m = dict(shared)
        m["xidx"] = np.ascontiguousarray(idx_np.transpose(1, 0, 2).reshape(128, -1))
        m["xmask"] = np.ascontiguousarray(msk_np.transpose(1, 0, 2).reshape(128, -1))
        in_maps.append(m)
    return in_maps


def _ensure_ntff_hook():
    """The image's antenv lacks axon_hooks; synthesize it so
    run_bass_kernel_spmd(trace=True) can NTFF-profile via the axon .so."""
    import types
    if 'antenv.axon_hooks' in sys.modules:
        return
    import antenv
    mod = types.ModuleType('antenv.axon_hooks')
    _state = {'hook': None}
    def set_axon_ntff_profile_hook(h):
        _state['hook'] = h
    def get_axon_ntff_profile_hook():
        if _state['hook'] is None:
            try:
                from trn_agent_boot.trn_boot import _ntff_profile_via_ctypes
                _state['hook'] = _ntff_profile_via_ctypes('/opt/axon/libaxon_pjrt.so')
            except Exception:
                return None
        return _state['hook']
    mod.set_axon_ntff_profile_hook = set_axon_ntff_profile_hook
    mod.get_axon_ntff_profile_hook = get_axon_ntff_profile_hook
    sys.modules['antenv.axon_hooks'] = mod
    antenv.axon_hooks = mod


def run(inputs: dict, trace: bool = False):
    if trace:
        _ensure_ntff_hook()
    nc = _get_program()
    in_maps = _host_prep(**inputs)
    res = run_bass_kernel_spmd(nc, in_maps, core_ids=list(range(NCORES)),
                               trace=trace)
    out = np.concatenate(
        [np.asarray(res.results[c]["out"]).reshape(BC, 1) for c in range(NCORES)]
    )
    return out.astype(np.float32), res


def kernel(**inputs):
    out, _ = run(inputs, trace=False)
    return out
